# revision 23
# baseline (speedup 1.0000x reference)
"""Trainium2 Bass kernel for an Aria-style MoE decoder layer (8-core SPMD).

Sharding:
  - Attention: head-parallel (20 heads -> 8 cores x 3 slots, 4 zero-padded),
    fp32r matmuls; o-projection partials combined with a fp32 ReduceScatter
    over the token axis (natural [T, D] layout).
  - Router/top-6: replicated per-token math on each core's 64-token slice,
    fp32; coefficients AllGathered.
  - Routed experts: expert-parallel, 2 experts/core, capacity 256/expert.
    Dispatch = one-hot gather matmul, combine = coefficient-weighted one-hot
    scatter matmul, all in bf16.
  - Shared expert: split along the intermediate dim (512 padded cols/core).
  - Final combine: bf16 ReduceScatter of MoE partials + local residual add.
"""

import numpy as np

import concourse.bass as bass
import concourse.mybir as mybir
import concourse.tile as tile
from concourse import bacc
from concourse.bass_utils import run_bass_kernel_spmd

try:
    import ml_dtypes
    ml_bf16 = ml_dtypes.bfloat16
except ImportError:  # pragma: no cover
    ml_bf16 = np.float16

F32 = mybir.dt.float32
F32R = mybir.dt.float32r
BF16 = mybir.dt.bfloat16
AF = mybir.ActivationFunctionType
ALU = mybir.AluOpType

NCORES = 8
T, D, NH, HD = 512, 2560, 20, 128
DT = D // 128            # 20 d-tiles
NSLOT = 3                # head slots per core (padded)
E, TOPK, EPC = 16, 6, 2  # experts, top-k, experts per core
I = 1664
IT = I // 128            # 13 i-tiles
C = 256                  # per-expert token capacity
SC = C // 128            # s-chunks per expert
SI = 512                 # shared-expert intermediate per core (416 padded)
SIT = SI // 128          # 4
TSL = T // NCORES        # 64 tokens per core slice
TCH = T // 128           # 4 token chunks
DCH = D // 512           # 5 d 512-chunks
EPS = 1e-6
ISQ = float(1.0 / np.sqrt(HD))

TRACE = False
_CACHE = {}


def _build():
    nc = bacc.Bacc("TRN2", target_bir_lowering=False, debug=False, num_devices=NCORES)

    def din(name, shape, dt):
        return nc.dram_tensor(name, shape, dt, kind="ExternalInput").ap()

    tn1T = din("tn1T", [128, DT, T], F32R)          # ln1-normed x, [dpart, dtile, tok]
    x_sl = din("x_sl", [TSL, D], F32)               # raw residual rows for this core
    wq = din("wq", [128, DT, NSLOT * 128], F32R)
    wk = din("wk", [128, DT, NSLOT * 128], F32R)
    wv = din("wv", [128, DT, NSLOT * 128], F32R)
    wo = din("wo", [128, NSLOT, D], F32R)
    cosT = din("cosT", [128, T], F32)
    sinT = din("sinT", [128, T], F32)               # rot-half signed sin, transposed
    maskT = din("maskT", [TCH, 128, T], F32)        # (mask & causal).T as [kc, krel, q]
    rw = din("rw", [128, DT, E], F32R)              # router weights (ln2 folded)
    sel = din("sel", [16, EPC], F32R)               # one-hot expert selector
    ones = din("ones", [128, 1], F32R)
    ident = din("ident", [128, 128], F32)
    pswap = din("pswap", [128, 128], F32R)          # rot-half permutation
    iotaC = din("iotaC", [128, C], F32)             # row p = [0..C-1]
    iotaS = din("iotaS", [128, SC], F32)            # col j = 128j + arange(128)
    wg = din("wg", [EPC, IT, 128, DT * 128], BF16)  # [e, it, dpart, (dtile,icol)]
    wu = din("wu", [EPC, IT, 128, DT * 128], BF16)
    wd = din("wd", [EPC, DCH, 128, IT * 512], BF16)  # [e, dc, ipart, (it,dcol)]
    swg = din("swg", [SIT, 128, DT * 128], BF16)    # [it, dpart, (dtile,icol)]
    swu = din("swu", [SIT, 128, DT * 128], BF16)
    swd = din("swd", [SIT, 128, D], BF16)           # [it, ipart, dcol]

    out_sl = nc.dram_tensor("out_sl", [TSL, D], F32, kind="ExternalOutput").ap()
    dbg_h = nc.dram_tensor("dbg_h", [TSL, D], F32, kind="ExternalOutput").ap()
    dbg_cw = nc.dram_tensor("dbg_cw", [T, E], F32, kind="ExternalOutput").ap()

    with tile.TileContext(nc) as tc:
      with tc.tile_pool(name="dram", bufs=1, space="DRAM") as dram, \
           tc.tile_pool(name="consts", bufs=1) as cpool:
        rs1_in = dram.tile([T, D], F32, tag="rs1i")
        rs1_out = dram.tile([TSL, D], F32, tag="rs1o")
        ag_tn_in = dram.tile([TSL, D], BF16, tag="agti")
        ag_tn_out = dram.tile([T, D], BF16, tag="agto")
        ag_cw_in = dram.tile([TSL, E], F32, tag="agci")
        ag_cw_out = dram.tile([T, E], F32, tag="agco")
        rs2_in = dram.tile([T, D], BF16, tag="rs2i")
        rs2_out = dram.tile([TSL, D], BF16, tag="rs2o")

        ones_sb = cpool.tile([128, 1], F32R, tag="ones")
        id_sb = cpool.tile([128, 128], F32, tag="id")
        nc.sync.dma_start(ones_sb[:], ones[:])
        nc.sync.dma_start(id_sb[:], ident[:])

        # ================= Phase A: attention =================
        with nc.named_scope("attn"), \
             tc.tile_pool(name="a_big", bufs=1) as abig, \
             tc.tile_pool(name="a_w", bufs=2) as awp, \
             tc.tile_pool(name="a_sb", bufs=1) as asb, \
             tc.tile_pool(name="a_tmp", bufs=2) as atmp:
            cos_sb = abig.tile([128, T], F32, tag="cos")
            sin_sb = abig.tile([128, T], F32, tag="sin")
            psw_sb = abig.tile([128, 128], F32R, tag="psw")
            nc.sync.dma_start(cos_sb[:], cosT[:])
            nc.sync.dma_start(sin_sb[:], sinT[:])
            nc.sync.dma_start(psw_sb[:], pswap[:])
            mask_sb = [abig.tile([128, T], F32, tag=f"mask{kc}", name=f"mask{kc}") for kc in range(TCH)]
            for kc in range(TCH):
                nc.sync.dma_start(mask_sb[kc][:], maskT[kc])
            tn1_sb = abig.tile([128, DT, T], F32R, tag="tn1")
            nc.sync.dma_start(tn1_sb[:], tn1T[:])

            # --- Q, K (transposed layout [hd, tok]), with rope ---
            qk_out = []
            with tc.tile_pool(name="qk_ps", bufs=6, space="PSUM") as qkps, \
                 tc.tile_pool(name="sw_ps", bufs=2, space="PSUM") as swps:
                for which, w_ap in (("q", wq), ("k", wk)):
                    psums = [qkps.tile([128, T], F32, tag="qk", name=f"qk{which}{i}") for i in range(NSLOT)]
                    for dt_i in range(DT):
                        wt = awp.tile([128, NSLOT * 128], F32R, tag="wqkv",
                                      name=f"w{which}{dt_i}")
                        nc.sync.dma_start(wt[:], w_ap[:, dt_i, :])
                        for s in range(NSLOT):
                            nc.tensor.matmul(
                                psums[s][:], wt[:, s * 128:(s + 1) * 128],
                                tn1_sb[:, dt_i, :],
                                start=(dt_i == 0), stop=(dt_i == DT - 1))
                    outs = []
                    for s in range(NSLOT):
                        # rope: out = q*cos + swap64(q)*sin_signed.
                        # swap64 is a cross-partition move -> PE permutation.
                        qs = atmp.tile([128, T], F32R, tag="qs")
                        nc.vector.tensor_copy(qs[:], psums[s][:])
                        swp = swps.tile([128, T], F32, tag="swp")
                        nc.tensor.matmul(swp[:], psw_sb[:], qs[:],
                                         start=True, stop=True)
                        t1 = atmp.tile([128, T], F32, tag="t1")
                        nc.vector.tensor_mul(t1[:], qs[:], cos_sb[:])
                        t2 = atmp.tile([128, T], F32, tag="t2")
                        nc.vector.tensor_mul(t2[:], swp[:], sin_sb[:])
                        o = asb.tile([128, T], F32R, tag=f"rope{which}{s}")
                        nc.vector.tensor_add(o[:], t1[:], t2[:])
                        outs.append(o)
                    qk_out.append(outs)
            qT, kT = qk_out

            # --- V (natural layout [tok, slot*128]) ---
            v_sb = []
            with tc.tile_pool(name="v_ps", bufs=4, space="PSUM") as vps:
                vp_l = [vps.tile([128, NSLOT * 128], F32, tag="vps",
                                 name=f"vp{i}") for i in range(TCH)]
                for dt_i in range(DT):
                    wvt = awp.tile([128, NSLOT * 128], F32R, tag="wqkv",
                                   name=f"wv{dt_i}")
                    nc.sync.dma_start(wvt[:], wv[:, dt_i, :])
                    for tc_i in range(TCH):
                        nc.tensor.matmul(
                            vp_l[tc_i][:],
                            tn1_sb[:, dt_i, tc_i * 128:(tc_i + 1) * 128],
                            wvt[:], start=(dt_i == 0), stop=(dt_i == DT - 1))
                for tc_i in range(TCH):
                    vs = asb.tile([128, NSLOT * 128], F32R, tag=f"v{tc_i}",
                                  name=f"v{tc_i}")
                    nc.vector.tensor_copy(vs[:], vp_l[tc_i][:])
                    v_sb.append(vs)

            # --- scores -> exp -> mask -> AV + denom, per slot ---
            ctx_n = []
            with tc.tile_pool(name="s_ps", bufs=2, space="PSUM") as sps, \
                 tc.tile_pool(name="c_ps", bufs=2, space="PSUM") as ctps, \
                 tc.tile_pool(name="dn_ps", bufs=2, space="PSUM") as dnps:
                for s in range(NSLOT):
                    ctxp = ctps.tile([128, T], F32, tag="ctx")
                    denp = dnps.tile([1, T], F32, tag="den")
                    for kc in range(TCH):
                        ncols = T - kc * 128
                        q0 = kc * 128
                        sp = sps.tile([128, T], F32, tag="scores")
                        nc.tensor.matmul(
                            sp[:, 0:ncols], kT[s][:, q0:q0 + 128], qT[s][:, q0:T],
                            start=True, stop=True)
                        ex = atmp.tile([128, T], F32R, tag="exp")
                        nc.scalar.activation(ex[:, 0:ncols], sp[:, 0:ncols], AF.Exp,
                                             scale=ISQ)
                        nc.vector.tensor_mul(ex[:, 0:ncols], ex[:, 0:ncols],
                                             mask_sb[kc][:, q0:T])
                        nc.tensor.matmul(
                            ctxp[:, q0:T], v_sb[kc][:, s * 128:(s + 1) * 128],
                            ex[:, 0:ncols], start=(kc == 0), stop=(kc == TCH - 1))
                        nc.tensor.matmul(
                            denp[:, q0:T], ones_sb[:], ex[:, 0:ncols],
                            start=(kc == 0), stop=(kc == TCH - 1))
                    rec = atmp.tile([1, T], F32, tag="rec")
                    nc.vector.reciprocal(rec[:], denp[:])
                    bc = atmp.tile([128, T], F32, tag="bc")
                    nc.gpsimd.partition_broadcast(bc[:], rec[:])
                    cn = asb.tile([128, T], F32R, tag=f"ctxn{s}")
                    nc.vector.tensor_mul(cn[:], ctxp[:], bc[:])
                    ctx_n.append(cn)

            # --- o-projection, natural [tok, d] output -> rs1_in (fp32) ---
            with tc.tile_pool(name="o_ps", bufs=2, space="PSUM") as ops_p:
                for dc in range(DCH):
                    wot = awp.tile([128, NSLOT, 512], F32R, tag="wo",
                                   name=f"wo{dc}")
                    nc.sync.dma_start(wot[:], wo[:, :, dc * 512:(dc + 1) * 512])
                    for tc_i in range(TCH):
                        op = ops_p.tile([128, 512], F32, tag="ops")
                        for s in range(NSLOT):
                            nc.tensor.matmul(
                                op[:], ctx_n[s][:, tc_i * 128:(tc_i + 1) * 128],
                                wot[:, s, :],
                                start=(s == 0), stop=(s == NSLOT - 1))
                        ob = atmp.tile([128, 512], F32, tag="ob")
                        nc.vector.tensor_copy(ob[:], op[:])
                        nc.sync.dma_start(
                            rs1_in[:].rearrange("(c p) d -> c p d", p=128)[tc_i, :, dc * 512:(dc + 1) * 512],
                            ob[:])

        nc.gpsimd.collective_compute(
            "ReduceScatter", ALU.add, replica_groups=[list(range(NCORES))],
            ins=[rs1_in.opt()], outs=[rs1_out.opt()])

        hpool_cm = tc.tile_pool(name="hpool", bufs=1)
        hpool = hpool_cm.__enter__()
        mpool_cm = tc.tile_pool(name="mpool", bufs=1)
        mpool = mpool_cm.__enter__()
        h_nat = hpool.tile([TSL, D], F32, tag="h")

        # ================= Phase B: h, ln2, router, top-k =================
        with nc.named_scope("router"), \
             tc.tile_pool(name="bwork", bufs=1) as bw, \
             tc.tile_pool(name="b_ps", bufs=2, space="PSUM") as bps, \
             tc.tile_pool(name="b_ps1", bufs=1, space="PSUM") as bps1:
            o_sl = bw.tile([TSL, D], F32, tag="osl")
            nc.sync.dma_start(o_sl[:], rs1_out[:])
            x_sb = bw.tile([TSL, D], F32, tag="xsl")
            nc.sync.dma_start(x_sb[:], x_sl[:])
            nc.vector.tensor_add(h_nat[:], x_sb[:], o_sl[:])
            nc.sync.dma_start(dbg_h[:], h_nat[:])

            sq = bw.tile([TSL, D], F32, tag="sq")
            ssq = bw.tile([TSL, 1], F32, tag="ssq")
            nc.scalar.activation(sq[:], h_nat[:], AF.Square, accum_out=ssq[:])
            eps_t = bw.tile([TSL, 1], F32, tag="epst")
            nc.gpsimd.memset(eps_t[:], EPS)
            rms = bw.tile([TSL, 1], F32, tag="rms")
            nc.scalar.activation(rms[:], ssq[:], AF.Sqrt, scale=float(1.0 / D),
                                 bias=eps_t[:])
            inv = bw.tile([TSL, 1], F32, tag="inv")
            nc.vector.reciprocal(inv[:], rms[:])
            t_sl = bw.tile([TSL, D], F32, tag="tsl")
            nc.vector.tensor_scalar_mul(t_sl[:], h_nat[:], inv[:])

            # bf16 copy of normalized slice -> AllGather input
            t_sl_bf = bw.tile([TSL, D], BF16, tag="tslbf")
            nc.vector.tensor_copy(t_sl_bf[:], t_sl[:])
            nc.sync.dma_start(ag_tn_in[:], t_sl_bf[:])

            # transpose slice -> tT_cols [d, tok] f32r for the router matmul
            rwt = bw.tile([128, DT, E], F32R, tag="rw")
            nc.sync.dma_start(rwt[:], rw[:])
            tts_l = []
            for dt_i in range(DT):
                tp = bps.tile([128, TSL], F32, tag="ttp")
                nc.tensor.transpose(tp[:], t_sl[:, dt_i * 128:(dt_i + 1) * 128],
                                    id_sb[0:TSL, 0:TSL])
                tts = bw.tile([128, TSL], F32R, tag=f"tts{dt_i}")
                nc.vector.tensor_copy(tts[:], tp[:])
                tts_l.append(tts)
            lg = bps1.tile([TSL, E], F32, tag="lg")
            for dt_i in range(DT):
                nc.tensor.matmul(lg[:], tts_l[dt_i][:], rwt[:, dt_i, :],
                                 start=(dt_i == 0), stop=(dt_i == DT - 1))

            # softmax over E (free dim)
            mx = bw.tile([TSL, 1], F32, tag="mx")
            nc.vector.tensor_reduce(mx[:], lg[:], mybir.AxisListType.X, ALU.max)
            nmx = bw.tile([TSL, 1], F32, tag="nmx")
            nc.vector.tensor_scalar_mul(nmx[:], mx[:], -1.0)
            ex = bw.tile([TSL, E], F32, tag="exr")
            sm = bw.tile([TSL, 1], F32, tag="smr")
            nc.scalar.activation(ex[:], lg[:], AF.Exp, bias=nmx[:], accum_out=sm[:])
            rs = bw.tile([TSL, 1], F32, tag="rsr")
            nc.vector.reciprocal(rs[:], sm[:])
            probs = bw.tile([TSL, E], F32, tag="probs")
            nc.vector.tensor_scalar_mul(probs[:], ex[:], rs[:])

            # iterative top-6: extract max 6 times
            work = bw.tile([TSL, E], F32, tag="work")
            nc.vector.tensor_copy(work[:], probs[:])
            tsum = bw.tile([TSL, 1], F32, tag="tsum")
            thr = bw.tile([TSL, 1], F32, tag="thr")
            for k in range(TOPK):
                m = bw.tile([TSL, 1], F32, tag=f"m{k}")
                nc.vector.tensor_reduce(m[:], work[:], mybir.AxisListType.X, ALU.max)
                if k == 0:
                    nc.vector.tensor_copy(tsum[:], m[:])
                else:
                    nc.vector.tensor_add(tsum[:], tsum[:], m[:])
                if k == TOPK - 1:
                    nc.vector.tensor_copy(thr[:], m[:])
                else:
                    eq = bw.tile([TSL, E], F32, tag="eq")
                    nc.vector.tensor_scalar(eq[:], work[:], m[:], None, op0=ALU.is_ge)
                    nc.vector.tensor_sub(work[:], work[:], eq[:])
            mask6 = bw.tile([TSL, E], F32, tag="mask6")
            nc.vector.tensor_scalar(mask6[:], probs[:], thr[:], None, op0=ALU.is_ge)
            cwu = bw.tile([TSL, E], F32, tag="cwu")
            nc.vector.tensor_mul(cwu[:], probs[:], mask6[:])
            rts = bw.tile([TSL, 1], F32, tag="rts")
            nc.vector.reciprocal(rts[:], tsum[:])
            cw = bw.tile([TSL, E], F32, tag="cw")
            nc.vector.tensor_scalar_mul(cw[:], cwu[:], rts[:])
            nc.sync.dma_start(ag_cw_in[:], cw[:])

        nc.gpsimd.collective_compute(
            "AllGather", ALU.bypass, replica_groups=[list(range(NCORES))],
            ins=[ag_tn_in.opt()], outs=[ag_tn_out.opt()])
        nc.gpsimd.collective_compute(
            "AllGather", ALU.bypass, replica_groups=[list(range(NCORES))],
            ins=[ag_cw_in.opt()], outs=[ag_cw_out.opt()])
        nc.sync.dma_start(dbg_cw[:], ag_cw_out[:])

        # ================= Phase C: dispatch prep =================
        # mpool holds what phase D needs: tT, tcT, STw, (later act/down tiles)
        tT = [mpool.tile([128, T], BF16, tag=f"tT{i}", name=f"tT{i}") for i in range(DT)]
        STw = {}
        tcT = {}
        with nc.named_scope("dispatch"), \
             tc.tile_pool(name="c_sb", bufs=1) as csb, \
             tc.tile_pool(name="c_tmp", bufs=2) as ctmp:
            tnat = []
            for tc_i in range(TCH):
                tt = csb.tile([128, D], BF16, tag=f"tnat{tc_i}")
                nc.sync.dma_start(
                    tt[:], ag_tn_out[:].rearrange("(c p) d -> c p d", p=128)[tc_i])
                tnat.append(tt)
            idb = csb.tile([128, 128], BF16, tag="idb")
            nc.vector.tensor_copy(idb[:], id_sb[:])
            with tc.tile_pool(name="ct_ps", bufs=4, space="PSUM") as ctp:
                for dt_i in range(DT):
                    for tc_i in range(TCH):
                        tp = ctp.tile([128, 128], BF16, tag="ttp")
                        nc.tensor.matmul(tp[:],
                                         tnat[tc_i][:, dt_i * 128:(dt_i + 1) * 128],
                                         idb[:], is_transpose=True,
                                         start=True, stop=True)
                        nc.vector.tensor_copy(
                            tT[dt_i][:, tc_i * 128:(tc_i + 1) * 128], tp[:])

            # cw -> cwT -> my 2 experts' rows; mask/pos/posm
            with tc.tile_pool(name="cw_ps", bufs=1, space="PSUM") as cwps, \
                 tc.tile_pool(name="cw_ps2", bufs=2, space="PSUM") as cwps2:
                cwn = []
                for tc_i in range(TCH):
                    cn = ctmp.tile([128, E], F32, tag="cwn")
                    nc.sync.dma_start(
                        cn[:], ag_cw_out[:].rearrange("(c p) e -> c p e", p=128)[tc_i])
                    cwn.append(cn)
                cwTp = cwps.tile([E, T], F32, tag="cwT")
                for tc_i in range(TCH):
                    nc.tensor.matmul(cwTp[:, tc_i * 128:(tc_i + 1) * 128],
                                     cwn[tc_i][:], id_sb[:], is_transpose=True,
                                     start=True, stop=True)
                cwT = csb.tile([E, T], F32R, tag="cwTs")
                nc.vector.tensor_copy(cwT[:], cwTp[:])
                selt = csb.tile([16, EPC], F32R, tag="sel")
                nc.sync.dma_start(selt[:], sel[:])
                zz = csb.tile([1, T], F32, tag="zz")
                nc.gpsimd.memset(zz[:], 0.0)
                pn = [csb.tile([128, EPC], F32, tag=f"pn{i}", name=f"pn{i}")
                      for i in range(TCH)]
                cwm = []
                posm = []
                for e in range(EPC):
                    # this expert's coefficient row, at partition 0
                    cwmp = cwps.tile([1, T], F32, tag=f"cwm{e}", name=f"cwmp{e}")
                    nc.tensor.matmul(cwmp[:], selt[:, e:e + 1], cwT[:],
                                     start=True, stop=True)
                    cwm_e = csb.tile([1, T], F32, tag=f"cwms{e}", name=f"cwms{e}")
                    nc.vector.tensor_copy(cwm_e[:], cwmp[:])
                    cwm.append(cwm_e)
                    msk = csb.tile([1, T], F32, tag=f"msk{e}", name=f"msk{e}")
                    nc.vector.tensor_scalar(msk[:], cwm_e[:], 0.0, None,
                                            op0=ALU.is_gt)
                    pos = csb.tile([1, T], F32, tag=f"pos{e}", name=f"pos{e}")
                    nc.vector.tensor_tensor_scan(pos[:], msk[:], zz[:], 0.0,
                                                 op0=ALU.add, op1=ALU.add)
                    pm = csb.tile([1, T], F32, tag=f"posm{e}", name=f"posm{e}")
                    nc.vector.tensor_mul(pm[:], pos[:], msk[:])
                    nc.vector.tensor_scalar_add(pm[:], pm[:], -1.0)
                    posm.append(pm)
                    # posm_nat [tok, 1] per token chunk -> pn[tc][:, e]
                    for tc_i in range(TCH):
                        pp = cwps2.tile([128, 1], F32, tag="pn")
                        nc.tensor.matmul(pp[:],
                                         pm[:, tc_i * 128:(tc_i + 1) * 128],
                                         id_sb[0:1, 0:1], is_transpose=True,
                                         start=True, stop=True)
                        nc.vector.tensor_copy(pn[tc_i][:, e:e + 1], pp[:])

            # S (gather) and cw-weighted ST (scatter) one-hot matrices
            iC = csb.tile([128, C], F32, tag="iC")
            nc.sync.dma_start(iC[:], iotaC[:])
            iS = csb.tile([128, SC], F32, tag="iS")
            nc.sync.dma_start(iS[:], iotaS[:])
            S = {}
            for e in range(EPC):
                for tc_i in range(TCH):
                    st = csb.tile([128, C], BF16, tag=f"S{e}_{tc_i}")
                    nc.vector.tensor_scalar(st[:], iC[:], pn[tc_i][:, e:e + 1], None,
                                            op0=ALU.is_equal)
                    S[(e, tc_i)] = st
                pb = ctmp.tile([128, T], F32, tag="pb")
                nc.gpsimd.partition_broadcast(pb[:], posm[e][:])
                cb = ctmp.tile([128, T], F32, tag="cb")
                nc.gpsimd.partition_broadcast(cb[:], cwm[e][:])
                for sc_i in range(SC):
                    t1 = ctmp.tile([128, T], F32, tag="st1")
                    nc.vector.tensor_scalar(t1[:], pb[:], iS[:, sc_i:sc_i + 1], None,
                                            op0=ALU.is_equal)
                    stw = mpool.tile([128, T], BF16, tag=f"STw{e}_{sc_i}")
                    nc.vector.tensor_mul(stw[:], t1[:], cb[:])
                    STw[(e, sc_i)] = stw

            # gather: tcT[e] tiles [128(d), C] bf16
            with tc.tile_pool(name="g_ps", bufs=3, space="PSUM") as gps_p:
                for e in range(EPC):
                    for dt_i in range(DT):
                        gp = gps_p.tile([128, C], F32, tag="gps")
                        for tc_i in range(TCH):
                            nc.tensor.matmul(
                                gp[:], tnat[tc_i][:, dt_i * 128:(dt_i + 1) * 128],
                                S[(e, tc_i)][:],
                                start=(tc_i == 0), stop=(tc_i == TCH - 1))
                        g = mpool.tile([128, C], BF16, tag=f"tcT{e}_{dt_i}")
                        nc.vector.tensor_copy(g[:], gp[:])
                        tcT[(e, dt_i)] = g

        # ================= Phase D: experts =================
        act = {}
        act_sh = []
        down = {}
        with nc.named_scope("experts"), \
             tc.tile_pool(name="d_w", bufs=4) as dwp, \
             tc.tile_pool(name="d_wd", bufs=2) as dwdp, \
             tc.tile_pool(name="d_sb", bufs=2) as dsb:
            with tc.tile_pool(name="gu_ps", bufs=2, space="PSUM") as gups:
                # routed gate/up -> act (bf16, kept in mpool until down)
                for e in range(EPC):
                    for it in range(IT):
                        wgt = dwp.tile([128, DT * 128], BF16, tag="wg")
                        nc.sync.dma_start(wgt[:], wg[e, it])
                        gp = gups.tile([128, T], F32, tag="gps")
                        for dt_i in range(DT):
                            nc.tensor.matmul(gp[:, 0:C],
                                             wgt[:, dt_i * 128:(dt_i + 1) * 128],
                                             tcT[(e, dt_i)][:],
                                             start=(dt_i == 0), stop=(dt_i == DT - 1))
                        wut = dwp.tile([128, DT * 128], BF16, tag="wu")
                        nc.sync.dma_start(wut[:], wu[e, it])
                        up = gups.tile([128, T], F32, tag="ups")
                        for dt_i in range(DT):
                            nc.tensor.matmul(up[:, 0:C],
                                             wut[:, dt_i * 128:(dt_i + 1) * 128],
                                             tcT[(e, dt_i)][:],
                                             start=(dt_i == 0), stop=(dt_i == DT - 1))
                        sl = dsb.tile([128, C], F32, tag="sl")
                        nc.scalar.activation(sl[:], gp[:, 0:C], AF.Silu)
                        a = mpool.tile([128, C], BF16, tag=f"act{e}_{it}")
                        nc.vector.tensor_mul(a[:], sl[:], up[:, 0:C])
                        act[(e, it)] = a

                # shared expert gate/up -> act_sh
                for it in range(SIT):
                    sgt = dwp.tile([128, DT * 128], BF16, tag="wg")
                    nc.sync.dma_start(sgt[:], swg[it])
                    gp = gups.tile([128, T], F32, tag="gps")
                    for dt_i in range(DT):
                        nc.tensor.matmul(gp[:], sgt[:, dt_i * 128:(dt_i + 1) * 128],
                                         tT[dt_i][:],
                                         start=(dt_i == 0), stop=(dt_i == DT - 1))
                    sut = dwp.tile([128, DT * 128], BF16, tag="wu")
                    nc.sync.dma_start(sut[:], swu[it])
                    up = gups.tile([128, T], F32, tag="ups")
                    for dt_i in range(DT):
                        nc.tensor.matmul(up[:], sut[:, dt_i * 128:(dt_i + 1) * 128],
                                         tT[dt_i][:],
                                         start=(dt_i == 0), stop=(dt_i == DT - 1))
                    sl = dsb.tile([128, T], F32, tag="ssl")
                    nc.scalar.activation(sl[:], gp[:], AF.Silu)
                    a = mpool.tile([128, T], BF16, tag=f"acts{it}")
                    nc.vector.tensor_mul(a[:], sl[:], up[:])
                    act_sh.append(a)

            # routed down: [s, d] = act.T @ wd, accumulated over i-tiles
            with tc.tile_pool(name="dn_ps", bufs=4, space="PSUM") as dnps:
                for e in range(EPC):
                    for dc in range(DCH):
                        wdt = dwdp.tile([128, IT * 512], BF16, tag="wd")
                        nc.sync.dma_start(wdt[:], wd[e, dc])
                        for sc_i in range(SC):
                            dp = dnps.tile([128, 512], F32, tag="dn")
                            for it in range(IT):
                                nc.tensor.matmul(
                                    dp[:],
                                    act[(e, it)][:, sc_i * 128:(sc_i + 1) * 128],
                                    wdt[:, it * 512:(it + 1) * 512],
                                    start=(it == 0), stop=(it == IT - 1))
                            db = mpool.tile([128, 512], BF16, tag=f"db{e}_{sc_i}_{dc}")
                            nc.vector.tensor_copy(db[:], dp[:])
                            down[(e, sc_i, dc)] = db

        # final: scatter routed + shared down -> rs2_in [T, D] bf16
        with nc.named_scope("combine"), \
             tc.tile_pool(name="f_w", bufs=1) as fwp, \
             tc.tile_pool(name="f_sb", bufs=3) as fsb, \
             tc.tile_pool(name="f_ps", bufs=3, space="PSUM") as fps_p:
            swd_sb = []
            for it in range(SIT):
                sdt = fwp.tile([128, D], BF16, tag=f"swd{it}")
                nc.sync.dma_start(sdt[:], swd[it])
                swd_sb.append(sdt)
            for tc_i in range(TCH):
                for dc in range(DCH):
                    fp = fps_p.tile([128, 512], F32, tag="fps")
                    for it in range(SIT):
                        nc.tensor.matmul(
                            fp[:], act_sh[it][:, tc_i * 128:(tc_i + 1) * 128],
                            swd_sb[it][:, dc * 512:(dc + 1) * 512],
                            start=(it == 0), stop=False)
                    n_sc = EPC * SC
                    cnt = 0
                    for e in range(EPC):
                        for sc_i in range(SC):
                            cnt += 1
                            nc.tensor.matmul(
                                fp[:],
                                STw[(e, sc_i)][:, tc_i * 128:(tc_i + 1) * 128],
                                down[(e, sc_i, dc)][:],
                                start=False, stop=(cnt == n_sc))
                    fb = fsb.tile([128, 512], BF16, tag="fb")
                    nc.vector.tensor_copy(fb[:], fp[:])
                    nc.sync.dma_start(
                        rs2_in[:].rearrange("(c p) d -> c p d", p=128)[tc_i, :, dc * 512:(dc + 1) * 512],
                        fb[:])

        nc.gpsimd.collective_compute(
            "ReduceScatter", ALU.add, replica_groups=[list(range(NCORES))],
            ins=[rs2_in.opt()], outs=[rs2_out.opt()])

        # ================= Final: residual add =================
        with tc.tile_pool(name="fin", bufs=1) as fin:
            moe_bf = fin.tile([TSL, D], BF16, tag="moebf")
            nc.sync.dma_start(moe_bf[:], rs2_out[:])
            moe_f = fin.tile([TSL, D], F32, tag="moef")
            nc.vector.tensor_copy(moe_f[:], moe_bf[:])
            o = fin.tile([TSL, D], F32, tag="o")
            nc.vector.tensor_add(o[:], h_nat[:], moe_f[:])
            nc.sync.dma_start(out_sl[:], o[:])

        mpool_cm.__exit__(None, None, None)
        hpool_cm.__exit__(None, None, None)

    nc.compile()
    return nc


def _prep_inputs(inputs):
    """Host-side prep: norms/tables/layout/sharding. Returns in_maps[8]."""
    f32 = np.float32
    x = np.asarray(inputs["hidden_states"], dtype=f32).reshape(T, D)
    ln1w = np.asarray(inputs["ln1_w"], dtype=f32)
    ln2w = np.asarray(inputs["ln2_w"], dtype=f32)
    pos_ids = np.asarray(inputs["position_ids"]).reshape(T)
    amask = np.asarray(inputs["attention_mask"]).reshape(T, T)

    xd = x.astype(np.float64)
    inv1 = 1.0 / np.sqrt((xd * xd).mean(axis=1, keepdims=True) + EPS)
    tn1 = ((xd * inv1).astype(f32)) * ln1w[None, :]
    tn1T_t = np.ascontiguousarray(
        tn1.T.reshape(DT, 128, T).transpose(1, 0, 2)).astype(f32)

    inv_freq = 1.0 / (5e6 ** (np.arange(0, HD, 2, dtype=f32) / HD))
    ang = pos_ids.astype(f32)[:, None] * inv_freq[None, :]        # [T, 64]
    cos_f = np.concatenate([np.cos(ang), np.cos(ang)], axis=1)    # [T, 128]
    sin_h = np.sin(ang)
    sinT_h = np.concatenate([-sin_h, sin_h], axis=1).T            # [128, T]
    cosT_h = np.ascontiguousarray(cos_f.T).astype(f32)
    sinT_h = np.ascontiguousarray(sinT_h).astype(f32)

    causal = np.tril(np.ones((T, T), dtype=bool))
    mk = (amask & causal).T.astype(f32)                           # [k, q]
    maskT_h = np.ascontiguousarray(mk.reshape(TCH, 128, T)).astype(f32)

    wq_f = np.asarray(inputs["wq"], dtype=f32)
    wk_f = np.asarray(inputs["wk"], dtype=f32)
    wv_f = np.asarray(inputs["wv"], dtype=f32)
    wo_f = np.asarray(inputs["wo"], dtype=f32)
    rw_f = np.asarray(inputs["router_w"], dtype=f32) * ln2w[:, None]
    wg_f = np.asarray(inputs["w_gate"], dtype=f32) * ln2w[None, :, None]
    wu_f = np.asarray(inputs["w_up"], dtype=f32) * ln2w[None, :, None]
    wd_f = np.asarray(inputs["w_down"], dtype=f32)
    sg_f = np.asarray(inputs["sw_gate"], dtype=f32) * ln2w[:, None]
    su_f = np.asarray(inputs["sw_up"], dtype=f32) * ln2w[:, None]
    sd_f = np.asarray(inputs["sw_down"], dtype=f32)

    rw_t = np.ascontiguousarray(rw_f.reshape(DT, 128, E).transpose(1, 0, 2))

    head_of = []
    for c in range(NCORES):
        if c < 4:
            head_of.append([3 * c, 3 * c + 1, 3 * c + 2])
        else:
            head_of.append([12 + 2 * (c - 4), 13 + 2 * (c - 4), None])

    iotaC_h = np.broadcast_to(np.arange(C, dtype=f32), (128, C)).copy()
    iotaS_h = (np.arange(128, dtype=f32)[:, None]
               + 128.0 * np.arange(SC, dtype=f32)[None, :]).copy()
    ident_h = np.eye(128, dtype=f32)
    ones_h = np.ones((128, 1), dtype=f32)
    # pswap[d, j] = 1 iff d == (j + 64) % 128, so (P.T @ q)[j] = q[(j+64)%128]
    pswap_h = np.zeros((128, 128), dtype=f32)
    for j in range(128):
        pswap_h[(j + 64) % 128, j] = 1.0

    sip = 3328 // NCORES  # 416

    def tile_gate(w):  # [D, SI] -> [SIT, 128, DT*128]
        return np.ascontiguousarray(
            w.reshape(DT, 128, SIT, 128).transpose(2, 1, 0, 3).reshape(
                SIT, 128, DT * 128))

    def exp_gate(w2):  # [D, I] -> [IT, 128, DT*128]
        return np.ascontiguousarray(
            w2.reshape(DT, 128, IT, 128).transpose(2, 1, 0, 3).reshape(
                IT, 128, DT * 128))

    def exp_down(w2):  # [I, D] -> [DCH, 128, IT*512]
        return np.ascontiguousarray(
            w2.reshape(IT, 128, DCH, 512).transpose(2, 1, 0, 3).reshape(
                DCH, 128, IT * 512))

    in_maps = []
    for c in range(NCORES):
        hs = head_of[c]
        wq_c = np.zeros((D, NSLOT * 128), f32)
        wk_c = np.zeros((D, NSLOT * 128), f32)
        wv_c = np.zeros((D, NSLOT * 128), f32)
        wo_c = np.zeros((NSLOT * 128, D), f32)
        for s, h in enumerate(hs):
            if h is None:
                continue
            wq_c[:, s * 128:(s + 1) * 128] = wq_f[:, h * 128:(h + 1) * 128]
            wk_c[:, s * 128:(s + 1) * 128] = wk_f[:, h * 128:(h + 1) * 128]
            wv_c[:, s * 128:(s + 1) * 128] = wv_f[:, h * 128:(h + 1) * 128]
            wo_c[s * 128:(s + 1) * 128, :] = wo_f[h * 128:(h + 1) * 128, :]

        def qt(w):  # [D, 384] -> [128, DT, 384]
            return np.ascontiguousarray(
                w.reshape(DT, 128, NSLOT * 128).transpose(1, 0, 2))

        wo_t = np.ascontiguousarray(wo_c.reshape(NSLOT, 128, D).transpose(1, 0, 2))

        e0, e1 = 2 * c, 2 * c + 1
        sel_h = np.zeros((16, EPC), f32)
        sel_h[e0, 0] = 1.0
        sel_h[e1, 1] = 1.0

        wg_c = np.stack([exp_gate(wg_f[e0]), exp_gate(wg_f[e1])]).astype(ml_bf16)
        wu_c = np.stack([exp_gate(wu_f[e0]), exp_gate(wu_f[e1])]).astype(ml_bf16)
        wd_c = np.stack([exp_down(wd_f[e0]), exp_down(wd_f[e1])]).astype(ml_bf16)

        s0 = c * sip
        sg_c = np.zeros((D, SI), f32)
        su_c = np.zeros((D, SI), f32)
        sd_c = np.zeros((SI, D), f32)
        sg_c[:, :sip] = sg_f[:, s0:s0 + sip]
        su_c[:, :sip] = su_f[:, s0:s0 + sip]
        sd_c[:sip, :] = sd_f[s0:s0 + sip, :]

        in_maps.append({
            "tn1T": tn1T_t,
            "x_sl": np.ascontiguousarray(x[c * TSL:(c + 1) * TSL]),
            "wq": qt(wq_c), "wk": qt(wk_c), "wv": qt(wv_c), "wo": wo_t,
            "cosT": cosT_h, "sinT": sinT_h, "maskT": maskT_h,
            "rw": rw_t, "sel": sel_h, "ones": ones_h, "ident": ident_h,
            "pswap": pswap_h,
            "iotaC": iotaC_h, "iotaS": iotaS_h,
            "wg": wg_c, "wu": wu_c, "wd": wd_c,
            "swg": tile_gate(sg_c).astype(ml_bf16),
            "swu": tile_gate(su_c).astype(ml_bf16),
            "swd": sd_c.reshape(SIT, 128, D).astype(ml_bf16),
        })
    return in_maps


def kernel(**inputs) -> np.ndarray:
    if "nc" not in _CACHE:
        _CACHE["nc"] = _build()
    nc = _CACHE["nc"]
    in_maps = _prep_inputs(inputs)
    res = run_bass_kernel_spmd(nc, in_maps, core_ids=list(range(NCORES)), trace=TRACE)
    _CACHE["last_results"] = res
    out = np.concatenate([res.results[c]["out_sl"] for c in range(NCORES)], axis=0)
    return out.reshape(1, T, D).astype(np.float32)


# revision 25
# speedup vs baseline: 1.0239x; 1.0239x over previous
"""Trainium2 Bass kernel for an Aria-style MoE decoder layer (8-core SPMD).

Sharding:
  - Attention: head-parallel (20 heads -> 8 cores x 3 slots, 4 zero-padded),
    fp32r matmuls; o-projection partials combined with a fp32 ReduceScatter
    over the token axis (natural [T, D] layout).
  - Router/top-6: replicated per-token math on each core's 64-token slice,
    fp32; coefficients AllGathered.
  - Routed experts: expert-parallel, 2 experts/core, capacity 256/expert.
    Dispatch = one-hot gather matmul, combine = coefficient-weighted one-hot
    scatter matmul, all in bf16.
  - Shared expert: split along the intermediate dim (512 padded cols/core).
  - Final combine: bf16 ReduceScatter of MoE partials + local residual add.
"""

import numpy as np

import concourse.bass as bass
import concourse.mybir as mybir
import concourse.tile as tile
from concourse import bacc
from concourse.bass_utils import run_bass_kernel_spmd

try:
    import ml_dtypes
    ml_bf16 = ml_dtypes.bfloat16
except ImportError:  # pragma: no cover
    ml_bf16 = np.float16

F32 = mybir.dt.float32
F32R = mybir.dt.float32r
BF16 = mybir.dt.bfloat16
AF = mybir.ActivationFunctionType
ALU = mybir.AluOpType

NCORES = 8
T, D, NH, HD = 512, 2560, 20, 128
DT = D // 128            # 20 d-tiles
NSLOT = 3                # head slots per core (padded)
E, TOPK, EPC = 16, 6, 2  # experts, top-k, experts per core
I = 1664
IT = I // 128            # 13 i-tiles
C = 256                  # per-expert token capacity
SC = C // 128            # s-chunks per expert
SI = 512                 # shared-expert intermediate per core (416 padded)
SIT = SI // 128          # 4
TSL = T // NCORES        # 64 tokens per core slice
TCH = T // 128           # 4 token chunks
DCH = D // 512           # 5 d 512-chunks
EPS = 1e-6
ISQ = float(1.0 / np.sqrt(HD))

TRACE = False
_CACHE = {}


def _build():
    nc = bacc.Bacc("TRN2", target_bir_lowering=False, debug=False, num_devices=NCORES)

    def din(name, shape, dt):
        return nc.dram_tensor(name, shape, dt, kind="ExternalInput").ap()

    tn1T = din("tn1T", [128, DT, T], F32R)          # ln1-normed x, [dpart, dtile, tok]
    x_sl = din("x_sl", [TSL, D], F32)               # raw residual rows for this core
    wq = din("wq", [128, DT, NSLOT * 128], F32R)
    wk = din("wk", [128, DT, NSLOT * 128], F32R)
    wv = din("wv", [128, DT, NSLOT * 128], F32R)
    wo = din("wo", [128, NSLOT, D], F32R)
    cosT = din("cosT", [128, T], F32)
    sinT = din("sinT", [128, T], F32)               # rot-half signed sin, transposed
    maskT = din("maskT", [TCH, 128, T], F32)        # (mask & causal).T as [kc, krel, q]
    rw = din("rw", [128, DT, E], F32R)              # router weights (ln2 folded)
    sel = din("sel", [16, EPC], F32R)               # one-hot expert selector
    ones = din("ones", [128, 1], F32R)
    ident = din("ident", [128, 128], F32)
    pswap = din("pswap", [128, 128], F32R)          # rot-half permutation
    iotaC = din("iotaC", [128, C], F32)             # row p = [0..C-1]
    iotaS = din("iotaS", [128, SC], F32)            # col j = 128j + arange(128)
    wg = din("wg", [EPC, IT, 128, DT * 128], BF16)  # [e, it, dpart, (dtile,icol)]
    wu = din("wu", [EPC, IT, 128, DT * 128], BF16)
    wd = din("wd", [EPC, DCH, 128, IT * 512], BF16)  # [e, dc, ipart, (it,dcol)]
    swg = din("swg", [SIT, 128, DT * 128], BF16)    # [it, dpart, (dtile,icol)]
    swu = din("swu", [SIT, 128, DT * 128], BF16)
    swd = din("swd", [SIT, 128, D], BF16)           # [it, ipart, dcol]

    out_sl = nc.dram_tensor("out_sl", [TSL, D], F32, kind="ExternalOutput").ap()
    dbg_h = nc.dram_tensor("dbg_h", [TSL, D], F32, kind="ExternalOutput").ap()
    dbg_cw = nc.dram_tensor("dbg_cw", [T, E], F32, kind="ExternalOutput").ap()

    with tile.TileContext(nc) as tc:
      with tc.tile_pool(name="dram", bufs=1, space="DRAM") as dram, \
           tc.tile_pool(name="consts", bufs=1) as cpool:
        rs1_in = dram.tile([T, D], F32, tag="rs1i")
        rs1_out = dram.tile([TSL, D], F32, tag="rs1o")
        ag_tn_in = dram.tile([TSL, D], BF16, tag="agti")
        ag_tn_out = dram.tile([T, D], BF16, tag="agto")
        ag_tT_in = dram.tile([DT, 128, TSL], BF16, tag="agtti")
        ag_tT_out = dram.tile([NCORES * DT, 128, TSL], BF16, tag="agtto")
        ag_cw_in = dram.tile([TSL, E], F32, tag="agci")
        ag_cw_out = dram.tile([T, E], F32, tag="agco")
        rs2_in = dram.tile([T, D], BF16, tag="rs2i")
        rs2_out = dram.tile([TSL, D], BF16, tag="rs2o")

        ones_sb = cpool.tile([128, 1], F32R, tag="ones")
        id_sb = cpool.tile([128, 128], F32, tag="id")
        nc.sync.dma_start(ones_sb[:], ones[:])
        nc.sync.dma_start(id_sb[:], ident[:])

        # ================= Phase A: attention =================
        with nc.named_scope("attn"), \
             tc.tile_pool(name="a_big", bufs=1) as abig, \
             tc.tile_pool(name="a_w", bufs=6) as awp, \
             tc.tile_pool(name="a_wo", bufs=2) as awop, \
             tc.tile_pool(name="a_sb", bufs=1) as asb, \
             tc.tile_pool(name="a_tmp", bufs=2) as atmp:
            cos_sb = abig.tile([128, T], F32, tag="cos")
            sin_sb = abig.tile([128, T], F32, tag="sin")
            psw_sb = abig.tile([128, 128], F32R, tag="psw")
            nc.sync.dma_start(cos_sb[:], cosT[:])
            nc.sync.dma_start(sin_sb[:], sinT[:])
            nc.sync.dma_start(psw_sb[:], pswap[:])
            mask_sb = [abig.tile([128, T], F32, tag=f"mask{kc}", name=f"mask{kc}") for kc in range(TCH)]
            for kc in range(TCH):
                nc.sync.dma_start(mask_sb[kc][:], maskT[kc])
            tn1_sb = abig.tile([128, DT, T], F32R, tag="tn1")
            for dt_i in range(DT):
                nc.sync.dma_start(tn1_sb[:, dt_i, :], tn1T[:, dt_i, :])

            # --- Q, K (transposed layout [hd, tok]), with rope ---
            qk_out = []
            with tc.tile_pool(name="qk_ps", bufs=6, space="PSUM") as qkps, \
                 tc.tile_pool(name="sw_ps", bufs=2, space="PSUM") as swps:
                for which, w_ap in (("q", wq), ("k", wk)):
                    psums = [qkps.tile([128, T], F32, tag="qk", name=f"qk{which}{i}") for i in range(NSLOT)]
                    for dt_i in range(DT):
                        wt = awp.tile([128, NSLOT * 128], F32R, tag="wqkv",
                                      name=f"w{which}{dt_i}")
                        nc.sync.dma_start(wt[:], w_ap[:, dt_i, :])
                        for s in range(NSLOT):
                            nc.tensor.matmul(
                                psums[s][:], wt[:, s * 128:(s + 1) * 128],
                                tn1_sb[:, dt_i, :],
                                start=(dt_i == 0), stop=(dt_i == DT - 1))
                    outs = []
                    for s in range(NSLOT):
                        # rope: out = q*cos + swap64(q)*sin_signed.
                        # swap64 is a cross-partition move -> PE permutation.
                        qs = atmp.tile([128, T], F32R, tag="qs")
                        nc.vector.tensor_copy(qs[:], psums[s][:])
                        swp = swps.tile([128, T], F32, tag="swp")
                        nc.tensor.matmul(swp[:], psw_sb[:], qs[:],
                                         start=True, stop=True)
                        t1 = atmp.tile([128, T], F32, tag="t1")
                        nc.vector.tensor_mul(t1[:], qs[:], cos_sb[:])
                        t2 = atmp.tile([128, T], F32, tag="t2")
                        nc.vector.tensor_mul(t2[:], swp[:], sin_sb[:])
                        o = asb.tile([128, T], F32R, tag=f"rope{which}{s}")
                        nc.vector.tensor_add(o[:], t1[:], t2[:])
                        outs.append(o)
                    qk_out.append(outs)
            qT, kT = qk_out

            # --- V (natural layout [tok, slot*128]) ---
            v_sb = []
            with tc.tile_pool(name="v_ps", bufs=4, space="PSUM") as vps:
                vp_l = [vps.tile([128, NSLOT * 128], F32, tag="vps",
                                 name=f"vp{i}") for i in range(TCH)]
                for dt_i in range(DT):
                    wvt = awp.tile([128, NSLOT * 128], F32R, tag="wqkv",
                                   name=f"wv{dt_i}")
                    nc.sync.dma_start(wvt[:], wv[:, dt_i, :])
                    for tc_i in range(TCH):
                        nc.tensor.matmul(
                            vp_l[tc_i][:],
                            tn1_sb[:, dt_i, tc_i * 128:(tc_i + 1) * 128],
                            wvt[:], start=(dt_i == 0), stop=(dt_i == DT - 1))
                for tc_i in range(TCH):
                    vs = asb.tile([128, NSLOT * 128], F32R, tag=f"v{tc_i}",
                                  name=f"v{tc_i}")
                    nc.vector.tensor_copy(vs[:], vp_l[tc_i][:])
                    v_sb.append(vs)

            # --- scores -> exp -> mask -> AV + denom, per slot ---
            ctx_n = []
            with tc.tile_pool(name="s_ps", bufs=2, space="PSUM") as sps, \
                 tc.tile_pool(name="c_ps", bufs=2, space="PSUM") as ctps, \
                 tc.tile_pool(name="dn_ps", bufs=2, space="PSUM") as dnps:
                for s in range(NSLOT):
                    ctxp = ctps.tile([128, T], F32, tag="ctx")
                    denp = dnps.tile([1, T], F32, tag="den")
                    for kc in range(TCH):
                        ncols = T - kc * 128
                        q0 = kc * 128
                        sp = sps.tile([128, T], F32, tag="scores")
                        nc.tensor.matmul(
                            sp[:, 0:ncols], kT[s][:, q0:q0 + 128], qT[s][:, q0:T],
                            start=True, stop=True)
                        ex = atmp.tile([128, T], F32R, tag="exp")
                        nc.scalar.activation(ex[:, 0:ncols], sp[:, 0:ncols], AF.Exp,
                                             scale=ISQ)
                        nc.vector.tensor_mul(ex[:, 0:ncols], ex[:, 0:ncols],
                                             mask_sb[kc][:, q0:T])
                        nc.tensor.matmul(
                            ctxp[:, q0:T], v_sb[kc][:, s * 128:(s + 1) * 128],
                            ex[:, 0:ncols], start=(kc == 0), stop=(kc == TCH - 1))
                        nc.tensor.matmul(
                            denp[:, q0:T], ones_sb[:], ex[:, 0:ncols],
                            start=(kc == 0), stop=(kc == TCH - 1))
                    rec = atmp.tile([1, T], F32, tag="rec")
                    nc.vector.reciprocal(rec[:], denp[:])
                    bc = atmp.tile([128, T], F32, tag="bc")
                    nc.gpsimd.partition_broadcast(bc[:], rec[:])
                    cn = asb.tile([128, T], F32R, tag=f"ctxn{s}")
                    nc.vector.tensor_mul(cn[:], ctxp[:], bc[:])
                    ctx_n.append(cn)

            # --- o-projection, natural [tok, d] output -> rs1_in (fp32) ---
            with tc.tile_pool(name="o_ps", bufs=2, space="PSUM") as ops_p:
                for dc in range(DCH):
                    wot = awop.tile([128, NSLOT, 512], F32R, tag="wo",
                                   name=f"wo{dc}")
                    nc.sync.dma_start(wot[:], wo[:, :, dc * 512:(dc + 1) * 512])
                    for tc_i in range(TCH):
                        op = ops_p.tile([128, 512], F32, tag="ops")
                        for s in range(NSLOT):
                            nc.tensor.matmul(
                                op[:], ctx_n[s][:, tc_i * 128:(tc_i + 1) * 128],
                                wot[:, s, :],
                                start=(s == 0), stop=(s == NSLOT - 1))
                        ob = atmp.tile([128, 512], F32, tag="ob")
                        nc.vector.tensor_copy(ob[:], op[:])
                        nc.sync.dma_start(
                            rs1_in[:].rearrange("(c p) d -> c p d", p=128)[tc_i, :, dc * 512:(dc + 1) * 512],
                            ob[:])

        nc.gpsimd.collective_compute(
            "ReduceScatter", ALU.add, replica_groups=[list(range(NCORES))],
            ins=[rs1_in.opt()], outs=[rs1_out.opt()])

        hpool_cm = tc.tile_pool(name="hpool", bufs=1)
        hpool = hpool_cm.__enter__()
        mpool_cm = tc.tile_pool(name="mpool", bufs=1)
        mpool = mpool_cm.__enter__()
        h_nat = hpool.tile([TSL, D], F32, tag="h")

        # ================= Phase B: h, ln2, router, top-k =================
        with nc.named_scope("router"), \
             tc.tile_pool(name="bwork", bufs=1) as bw, \
             tc.tile_pool(name="b_ps", bufs=2, space="PSUM") as bps, \
             tc.tile_pool(name="b_ps1", bufs=1, space="PSUM") as bps1:
            o_sl = bw.tile([TSL, D], F32, tag="osl")
            nc.sync.dma_start(o_sl[:], rs1_out[:])
            x_sb = bw.tile([TSL, D], F32, tag="xsl")
            nc.sync.dma_start(x_sb[:], x_sl[:])
            nc.vector.tensor_add(h_nat[:], x_sb[:], o_sl[:])
            nc.sync.dma_start(dbg_h[:], h_nat[:])

            sq = bw.tile([TSL, D], F32, tag="sq")
            ssq = bw.tile([TSL, 1], F32, tag="ssq")
            nc.scalar.activation(sq[:], h_nat[:], AF.Square, accum_out=ssq[:])
            eps_t = bw.tile([TSL, 1], F32, tag="epst")
            nc.gpsimd.memset(eps_t[:], EPS)
            rms = bw.tile([TSL, 1], F32, tag="rms")
            nc.scalar.activation(rms[:], ssq[:], AF.Sqrt, scale=float(1.0 / D),
                                 bias=eps_t[:])
            inv = bw.tile([TSL, 1], F32, tag="inv")
            nc.vector.reciprocal(inv[:], rms[:])
            t_sl = bw.tile([TSL, D], F32, tag="tsl")
            nc.vector.tensor_scalar_mul(t_sl[:], h_nat[:], inv[:])

            # bf16 copy of normalized slice -> AllGather input
            t_sl_bf = bw.tile([TSL, D], BF16, tag="tslbf")
            nc.vector.tensor_copy(t_sl_bf[:], t_sl[:])
            nc.sync.dma_start(ag_tn_in[:], t_sl_bf[:])

            # transpose slice -> tT_cols [d, tok] f32r for the router matmul
            rwt = bw.tile([128, DT, E], F32R, tag="rw")
            nc.sync.dma_start(rwt[:], rw[:])
            tts_l = []
            for dt_i in range(DT):
                tp = bps.tile([128, TSL], F32, tag="ttp")
                nc.tensor.transpose(tp[:], t_sl[:, dt_i * 128:(dt_i + 1) * 128],
                                    id_sb[0:TSL, 0:TSL])
                tts = bw.tile([128, TSL], F32R, tag=f"tts{dt_i}")
                nc.vector.tensor_copy(tts[:], tp[:])
                tts_l.append(tts)
            lg = bps1.tile([TSL, E], F32, tag="lg")
            for dt_i in range(DT):
                nc.tensor.matmul(lg[:], tts_l[dt_i][:], rwt[:, dt_i, :],
                                 start=(dt_i == 0), stop=(dt_i == DT - 1))
            ttb = bw.tile([128, DT, TSL], BF16, tag="ttb")
            for dt_i in range(DT):
                nc.vector.tensor_copy(ttb[:, dt_i, :], tts_l[dt_i][:])
            nc.sync.dma_start(
                ag_tT_in[:].rearrange("dt p t -> p dt t"), ttb[:])

            # top-k on unnormalized exp(logits): same selection and, since
            # cw = top6 / sum(top6), the softmax denominator cancels exactly.
            ex = bw.tile([TSL, E], F32, tag="exr")
            nc.scalar.activation(ex[:], lg[:], AF.Exp)
            probs = ex

            # iterative top-6: extract max 6 times
            work = bw.tile([TSL, E], F32, tag="work")
            nc.vector.tensor_copy(work[:], probs[:])
            tsum = bw.tile([TSL, 1], F32, tag="tsum")
            thr = bw.tile([TSL, 1], F32, tag="thr")
            for k in range(TOPK):
                m = bw.tile([TSL, 1], F32, tag=f"m{k}")
                nc.vector.tensor_reduce(m[:], work[:], mybir.AxisListType.X, ALU.max)
                if k == 0:
                    nc.vector.tensor_copy(tsum[:], m[:])
                else:
                    nc.vector.tensor_add(tsum[:], tsum[:], m[:])
                if k == TOPK - 1:
                    nc.vector.tensor_copy(thr[:], m[:])
                else:
                    eq = bw.tile([TSL, E], F32, tag="eq")
                    nc.vector.tensor_scalar(eq[:], work[:], m[:], 1e9,
                                            op0=ALU.is_ge, op1=ALU.mult)
                    nc.vector.tensor_sub(work[:], work[:], eq[:])
            mask6 = bw.tile([TSL, E], F32, tag="mask6")
            nc.vector.tensor_scalar(mask6[:], probs[:], thr[:], None, op0=ALU.is_ge)
            cwu = bw.tile([TSL, E], F32, tag="cwu")
            nc.vector.tensor_mul(cwu[:], probs[:], mask6[:])
            rts = bw.tile([TSL, 1], F32, tag="rts")
            nc.vector.reciprocal(rts[:], tsum[:])
            cw = bw.tile([TSL, E], F32, tag="cw")
            nc.vector.tensor_scalar_mul(cw[:], cwu[:], rts[:])
            nc.sync.dma_start(ag_cw_in[:], cw[:])

        nc.gpsimd.collective_compute(
            "AllGather", ALU.bypass, replica_groups=[list(range(NCORES))],
            ins=[ag_tn_in.opt()], outs=[ag_tn_out.opt()])
        nc.gpsimd.collective_compute(
            "AllGather", ALU.bypass, replica_groups=[list(range(NCORES))],
            ins=[ag_cw_in.opt()], outs=[ag_cw_out.opt()])
        nc.gpsimd.collective_compute(
            "AllGather", ALU.bypass, replica_groups=[list(range(NCORES))],
            ins=[ag_tT_in.opt()], outs=[ag_tT_out.opt()])
        nc.sync.dma_start(dbg_cw[:], ag_cw_out[:])

        # ================= Phase C: dispatch prep =================
        # mpool holds what phase D needs: tT, tcT, STw, (later act/down tiles)
        tT = [mpool.tile([128, T], BF16, tag=f"tT{i}", name=f"tT{i}") for i in range(DT)]
        STw = {}
        tcT = {}
        with nc.named_scope("dispatch"), \
             tc.tile_pool(name="c_sb", bufs=1) as csb, \
             tc.tile_pool(name="c_tmp", bufs=2) as ctmp:
            tnat = []
            for tc_i in range(TCH):
                tt = csb.tile([128, D], BF16, tag=f"tnat{tc_i}")
                nc.sync.dma_start(
                    tt[:], ag_tn_out[:].rearrange("(c p) d -> c p d", p=128)[tc_i])
                tnat.append(tt)
            for dt_i in range(DT):
                for c in range(NCORES):
                    nc.sync.dma_start(tT[dt_i][:, c * TSL:(c + 1) * TSL],
                                      ag_tT_out[c * DT + dt_i])

            # cw -> cwT -> my 2 experts' rows; mask/pos/posm
            with tc.tile_pool(name="cw_ps", bufs=1, space="PSUM") as cwps, \
                 tc.tile_pool(name="cw_ps2", bufs=2, space="PSUM") as cwps2:
                cwn = []
                for tc_i in range(TCH):
                    cn = ctmp.tile([128, E], F32, tag="cwn")
                    nc.sync.dma_start(
                        cn[:], ag_cw_out[:].rearrange("(c p) e -> c p e", p=128)[tc_i])
                    cwn.append(cn)
                cwTp = cwps.tile([E, T], F32, tag="cwT")
                for tc_i in range(TCH):
                    nc.tensor.matmul(cwTp[:, tc_i * 128:(tc_i + 1) * 128],
                                     cwn[tc_i][:], id_sb[:], is_transpose=True,
                                     start=True, stop=True)
                cwT = csb.tile([E, T], F32R, tag="cwTs")
                nc.vector.tensor_copy(cwT[:], cwTp[:])
                selt = csb.tile([16, EPC], F32R, tag="sel")
                nc.sync.dma_start(selt[:], sel[:])
                zz = csb.tile([1, T], F32, tag="zz")
                nc.gpsimd.memset(zz[:], 0.0)
                pn = [csb.tile([128, EPC], F32, tag=f"pn{i}", name=f"pn{i}")
                      for i in range(TCH)]
                cwm = []
                posm = []
                for e in range(EPC):
                    # this expert's coefficient row, at partition 0
                    cwmp = cwps.tile([1, T], F32, tag=f"cwm{e}", name=f"cwmp{e}")
                    nc.tensor.matmul(cwmp[:], selt[:, e:e + 1], cwT[:],
                                     start=True, stop=True)
                    cwm_e = csb.tile([1, T], F32, tag=f"cwms{e}", name=f"cwms{e}")
                    nc.vector.tensor_copy(cwm_e[:], cwmp[:])
                    cwm.append(cwm_e)
                    msk = csb.tile([1, T], F32, tag=f"msk{e}", name=f"msk{e}")
                    nc.vector.tensor_scalar(msk[:], cwm_e[:], 0.0, None,
                                            op0=ALU.is_gt)
                    pos = csb.tile([1, T], F32, tag=f"pos{e}", name=f"pos{e}")
                    nc.vector.tensor_tensor_scan(pos[:], msk[:], zz[:], 0.0,
                                                 op0=ALU.add, op1=ALU.add)
                    pm = csb.tile([1, T], F32, tag=f"posm{e}", name=f"posm{e}")
                    nc.vector.tensor_mul(pm[:], pos[:], msk[:])
                    nc.vector.tensor_scalar_add(pm[:], pm[:], -1.0)
                    posm.append(pm)
                    # posm_nat [tok, 1] per token chunk -> pn[tc][:, e]
                    for tc_i in range(TCH):
                        pp = cwps2.tile([128, 1], F32, tag="pn")
                        nc.tensor.matmul(pp[:],
                                         pm[:, tc_i * 128:(tc_i + 1) * 128],
                                         id_sb[0:1, 0:1], is_transpose=True,
                                         start=True, stop=True)
                        nc.vector.tensor_copy(pn[tc_i][:, e:e + 1], pp[:])

            # S (gather) and cw-weighted ST (scatter) one-hot matrices
            iC = csb.tile([128, C], F32, tag="iC")
            nc.sync.dma_start(iC[:], iotaC[:])
            iS = csb.tile([128, SC], F32, tag="iS")
            nc.sync.dma_start(iS[:], iotaS[:])
            S = {}
            for e in range(EPC):
                for tc_i in range(TCH):
                    st = csb.tile([128, C], BF16, tag=f"S{e}_{tc_i}")
                    nc.vector.tensor_scalar(st[:], iC[:], pn[tc_i][:, e:e + 1], None,
                                            op0=ALU.is_equal)
                    S[(e, tc_i)] = st
                pb = ctmp.tile([128, T], F32, tag="pb")
                nc.gpsimd.partition_broadcast(pb[:], posm[e][:])
                cb = ctmp.tile([128, T], F32, tag="cb")
                nc.gpsimd.partition_broadcast(cb[:], cwm[e][:])
                for sc_i in range(SC):
                    t1 = ctmp.tile([128, T], F32, tag="st1")
                    nc.vector.tensor_scalar(t1[:], pb[:], iS[:, sc_i:sc_i + 1], None,
                                            op0=ALU.is_equal)
                    stw = mpool.tile([128, T], BF16, tag=f"STw{e}_{sc_i}")
                    nc.vector.tensor_mul(stw[:], t1[:], cb[:])
                    STw[(e, sc_i)] = stw

            # gather: tcT[e] tiles [128(d), C] bf16
            with tc.tile_pool(name="g_ps", bufs=3, space="PSUM") as gps_p:
                for e in range(EPC):
                    for dt_i in range(DT):
                        gp = gps_p.tile([128, C], F32, tag="gps")
                        for tc_i in range(TCH):
                            nc.tensor.matmul(
                                gp[:], tnat[tc_i][:, dt_i * 128:(dt_i + 1) * 128],
                                S[(e, tc_i)][:],
                                start=(tc_i == 0), stop=(tc_i == TCH - 1))
                        g = mpool.tile([128, C], BF16, tag=f"tcT{e}_{dt_i}")
                        nc.vector.tensor_copy(g[:], gp[:])
                        tcT[(e, dt_i)] = g

        # ================= Phase D: experts =================
        act = {}
        act_sh = []
        down = {}
        with nc.named_scope("experts"), \
             tc.tile_pool(name="d_w", bufs=5) as dwp, \
             tc.tile_pool(name="d_wd", bufs=2) as dwdp, \
             tc.tile_pool(name="d_sb", bufs=2) as dsb:
            with tc.tile_pool(name="gu_ps", bufs=2, space="PSUM") as gups:
                # routed gate/up -> act (bf16, kept in mpool until down)
                for e in range(EPC):
                    for it in range(IT):
                        wgt = dwp.tile([128, DT * 128], BF16, tag="wg")
                        nc.sync.dma_start(wgt[:], wg[e, it])
                        gp = gups.tile([128, T], F32, tag="gps")
                        for dt_i in range(DT):
                            nc.tensor.matmul(gp[:, 0:C],
                                             wgt[:, dt_i * 128:(dt_i + 1) * 128],
                                             tcT[(e, dt_i)][:],
                                             start=(dt_i == 0), stop=(dt_i == DT - 1))
                        wut = dwp.tile([128, DT * 128], BF16, tag="wu")
                        nc.sync.dma_start(wut[:], wu[e, it])
                        up = gups.tile([128, T], F32, tag="ups")
                        for dt_i in range(DT):
                            nc.tensor.matmul(up[:, 0:C],
                                             wut[:, dt_i * 128:(dt_i + 1) * 128],
                                             tcT[(e, dt_i)][:],
                                             start=(dt_i == 0), stop=(dt_i == DT - 1))
                        sl = dsb.tile([128, C], F32, tag="sl")
                        nc.scalar.activation(sl[:], gp[:, 0:C], AF.Silu)
                        a = mpool.tile([128, C], BF16, tag=f"act{e}_{it}")
                        nc.vector.tensor_mul(a[:], sl[:], up[:, 0:C])
                        act[(e, it)] = a

                # shared expert gate/up -> act_sh
                for it in range(SIT):
                    sgt = dwp.tile([128, DT * 128], BF16, tag="wg")
                    nc.sync.dma_start(sgt[:], swg[it])
                    gp = gups.tile([128, T], F32, tag="gps")
                    for dt_i in range(DT):
                        nc.tensor.matmul(gp[:], sgt[:, dt_i * 128:(dt_i + 1) * 128],
                                         tT[dt_i][:],
                                         start=(dt_i == 0), stop=(dt_i == DT - 1))
                    sut = dwp.tile([128, DT * 128], BF16, tag="wu")
                    nc.sync.dma_start(sut[:], swu[it])
                    up = gups.tile([128, T], F32, tag="ups")
                    for dt_i in range(DT):
                        nc.tensor.matmul(up[:], sut[:, dt_i * 128:(dt_i + 1) * 128],
                                         tT[dt_i][:],
                                         start=(dt_i == 0), stop=(dt_i == DT - 1))
                    sl = dsb.tile([128, T], F32, tag="ssl")
                    nc.scalar.activation(sl[:], gp[:], AF.Silu)
                    a = mpool.tile([128, T], BF16, tag=f"acts{it}")
                    nc.vector.tensor_mul(a[:], sl[:], up[:])
                    act_sh.append(a)

            # routed down: [s, d] = act.T @ wd, accumulated over i-tiles
            with tc.tile_pool(name="dn_ps", bufs=4, space="PSUM") as dnps:
                for e in range(EPC):
                    for dc in range(DCH):
                        wdt = dwdp.tile([128, IT * 512], BF16, tag="wd")
                        nc.sync.dma_start(wdt[:], wd[e, dc])
                        for sc_i in range(SC):
                            dp = dnps.tile([128, 512], F32, tag="dn")
                            for it in range(IT):
                                nc.tensor.matmul(
                                    dp[:],
                                    act[(e, it)][:, sc_i * 128:(sc_i + 1) * 128],
                                    wdt[:, it * 512:(it + 1) * 512],
                                    start=(it == 0), stop=(it == IT - 1))
                            db = mpool.tile([128, 512], BF16, tag=f"db{e}_{sc_i}_{dc}")
                            nc.vector.tensor_copy(db[:], dp[:])
                            down[(e, sc_i, dc)] = db

        # final: scatter routed + shared down -> rs2_in [T, D] bf16
        with nc.named_scope("combine"), \
             tc.tile_pool(name="f_w", bufs=1) as fwp, \
             tc.tile_pool(name="f_sb", bufs=3) as fsb, \
             tc.tile_pool(name="f_ps", bufs=3, space="PSUM") as fps_p:
            swd_sb = []
            for it in range(SIT):
                sdt = fwp.tile([128, D], BF16, tag=f"swd{it}")
                nc.sync.dma_start(sdt[:], swd[it])
                swd_sb.append(sdt)
            for tc_i in range(TCH):
                for dc in range(DCH):
                    fp = fps_p.tile([128, 512], F32, tag="fps")
                    for it in range(SIT):
                        nc.tensor.matmul(
                            fp[:], act_sh[it][:, tc_i * 128:(tc_i + 1) * 128],
                            swd_sb[it][:, dc * 512:(dc + 1) * 512],
                            start=(it == 0), stop=False)
                    n_sc = EPC * SC
                    cnt = 0
                    for e in range(EPC):
                        for sc_i in range(SC):
                            cnt += 1
                            nc.tensor.matmul(
                                fp[:],
                                STw[(e, sc_i)][:, tc_i * 128:(tc_i + 1) * 128],
                                down[(e, sc_i, dc)][:],
                                start=False, stop=(cnt == n_sc))
                    fb = fsb.tile([128, 512], BF16, tag="fb")
                    nc.vector.tensor_copy(fb[:], fp[:])
                    nc.sync.dma_start(
                        rs2_in[:].rearrange("(c p) d -> c p d", p=128)[tc_i, :, dc * 512:(dc + 1) * 512],
                        fb[:])

        nc.gpsimd.collective_compute(
            "ReduceScatter", ALU.add, replica_groups=[list(range(NCORES))],
            ins=[rs2_in.opt()], outs=[rs2_out.opt()])

        # ================= Final: residual add =================
        with tc.tile_pool(name="fin", bufs=1) as fin:
            moe_bf = fin.tile([TSL, D], BF16, tag="moebf")
            nc.sync.dma_start(moe_bf[:], rs2_out[:])
            moe_f = fin.tile([TSL, D], F32, tag="moef")
            nc.vector.tensor_copy(moe_f[:], moe_bf[:])
            o = fin.tile([TSL, D], F32, tag="o")
            nc.vector.tensor_add(o[:], h_nat[:], moe_f[:])
            nc.sync.dma_start(out_sl[:], o[:])

        mpool_cm.__exit__(None, None, None)
        hpool_cm.__exit__(None, None, None)

    nc.compile()
    return nc


def _prep_inputs(inputs):
    """Host-side prep: norms/tables/layout/sharding. Returns in_maps[8]."""
    f32 = np.float32
    x = np.asarray(inputs["hidden_states"], dtype=f32).reshape(T, D)
    ln1w = np.asarray(inputs["ln1_w"], dtype=f32)
    ln2w = np.asarray(inputs["ln2_w"], dtype=f32)
    pos_ids = np.asarray(inputs["position_ids"]).reshape(T)
    amask = np.asarray(inputs["attention_mask"]).reshape(T, T)

    xd = x.astype(np.float64)
    inv1 = 1.0 / np.sqrt((xd * xd).mean(axis=1, keepdims=True) + EPS)
    tn1 = ((xd * inv1).astype(f32)) * ln1w[None, :]
    tn1T_t = np.ascontiguousarray(
        tn1.T.reshape(DT, 128, T).transpose(1, 0, 2)).astype(f32)

    inv_freq = 1.0 / (5e6 ** (np.arange(0, HD, 2, dtype=f32) / HD))
    ang = pos_ids.astype(f32)[:, None] * inv_freq[None, :]        # [T, 64]
    cos_f = np.concatenate([np.cos(ang), np.cos(ang)], axis=1)    # [T, 128]
    sin_h = np.sin(ang)
    sinT_h = np.concatenate([-sin_h, sin_h], axis=1).T            # [128, T]
    cosT_h = np.ascontiguousarray(cos_f.T).astype(f32)
    sinT_h = np.ascontiguousarray(sinT_h).astype(f32)

    causal = np.tril(np.ones((T, T), dtype=bool))
    mk = (amask & causal).T.astype(f32)                           # [k, q]
    maskT_h = np.ascontiguousarray(mk.reshape(TCH, 128, T)).astype(f32)

    wq_f = np.asarray(inputs["wq"], dtype=f32)
    wk_f = np.asarray(inputs["wk"], dtype=f32)
    wv_f = np.asarray(inputs["wv"], dtype=f32)
    wo_f = np.asarray(inputs["wo"], dtype=f32)
    rw_f = np.asarray(inputs["router_w"], dtype=f32) * ln2w[:, None]
    wg_f = np.asarray(inputs["w_gate"], dtype=f32) * ln2w[None, :, None]
    wu_f = np.asarray(inputs["w_up"], dtype=f32) * ln2w[None, :, None]
    wd_f = np.asarray(inputs["w_down"], dtype=f32)
    sg_f = np.asarray(inputs["sw_gate"], dtype=f32) * ln2w[:, None]
    su_f = np.asarray(inputs["sw_up"], dtype=f32) * ln2w[:, None]
    sd_f = np.asarray(inputs["sw_down"], dtype=f32)

    rw_t = np.ascontiguousarray(rw_f.reshape(DT, 128, E).transpose(1, 0, 2))

    head_of = []
    for c in range(NCORES):
        if c < 4:
            head_of.append([3 * c, 3 * c + 1, 3 * c + 2])
        else:
            head_of.append([12 + 2 * (c - 4), 13 + 2 * (c - 4), None])

    iotaC_h = np.broadcast_to(np.arange(C, dtype=f32), (128, C)).copy()
    iotaS_h = (np.arange(128, dtype=f32)[:, None]
               + 128.0 * np.arange(SC, dtype=f32)[None, :]).copy()
    ident_h = np.eye(128, dtype=f32)
    ones_h = np.ones((128, 1), dtype=f32)
    # pswap[d, j] = 1 iff d == (j + 64) % 128, so (P.T @ q)[j] = q[(j+64)%128]
    pswap_h = np.zeros((128, 128), dtype=f32)
    for j in range(128):
        pswap_h[(j + 64) % 128, j] = 1.0

    sip = 3328 // NCORES  # 416

    def tile_gate(w):  # [D, SI] -> [SIT, 128, DT*128]
        return np.ascontiguousarray(
            w.reshape(DT, 128, SIT, 128).transpose(2, 1, 0, 3).reshape(
                SIT, 128, DT * 128))

    def exp_gate(w2):  # [D, I] -> [IT, 128, DT*128]
        return np.ascontiguousarray(
            w2.reshape(DT, 128, IT, 128).transpose(2, 1, 0, 3).reshape(
                IT, 128, DT * 128))

    def exp_down(w2):  # [I, D] -> [DCH, 128, IT*512]
        return np.ascontiguousarray(
            w2.reshape(IT, 128, DCH, 512).transpose(2, 1, 0, 3).reshape(
                DCH, 128, IT * 512))

    in_maps = []
    for c in range(NCORES):
        hs = head_of[c]
        wq_c = np.zeros((D, NSLOT * 128), f32)
        wk_c = np.zeros((D, NSLOT * 128), f32)
        wv_c = np.zeros((D, NSLOT * 128), f32)
        wo_c = np.zeros((NSLOT * 128, D), f32)
        for s, h in enumerate(hs):
            if h is None:
                continue
            wq_c[:, s * 128:(s + 1) * 128] = wq_f[:, h * 128:(h + 1) * 128]
            wk_c[:, s * 128:(s + 1) * 128] = wk_f[:, h * 128:(h + 1) * 128]
            wv_c[:, s * 128:(s + 1) * 128] = wv_f[:, h * 128:(h + 1) * 128]
            wo_c[s * 128:(s + 1) * 128, :] = wo_f[h * 128:(h + 1) * 128, :]

        def qt(w):  # [D, 384] -> [128, DT, 384]
            return np.ascontiguousarray(
                w.reshape(DT, 128, NSLOT * 128).transpose(1, 0, 2))

        wo_t = np.ascontiguousarray(wo_c.reshape(NSLOT, 128, D).transpose(1, 0, 2))

        e0, e1 = 2 * c, 2 * c + 1
        sel_h = np.zeros((16, EPC), f32)
        sel_h[e0, 0] = 1.0
        sel_h[e1, 1] = 1.0

        wg_c = np.stack([exp_gate(wg_f[e0]), exp_gate(wg_f[e1])]).astype(ml_bf16)
        wu_c = np.stack([exp_gate(wu_f[e0]), exp_gate(wu_f[e1])]).astype(ml_bf16)
        wd_c = np.stack([exp_down(wd_f[e0]), exp_down(wd_f[e1])]).astype(ml_bf16)

        s0 = c * sip
        sg_c = np.zeros((D, SI), f32)
        su_c = np.zeros((D, SI), f32)
        sd_c = np.zeros((SI, D), f32)
        sg_c[:, :sip] = sg_f[:, s0:s0 + sip]
        su_c[:, :sip] = su_f[:, s0:s0 + sip]
        sd_c[:sip, :] = sd_f[s0:s0 + sip, :]

        in_maps.append({
            "tn1T": tn1T_t,
            "x_sl": np.ascontiguousarray(x[c * TSL:(c + 1) * TSL]),
            "wq": qt(wq_c), "wk": qt(wk_c), "wv": qt(wv_c), "wo": wo_t,
            "cosT": cosT_h, "sinT": sinT_h, "maskT": maskT_h,
            "rw": rw_t, "sel": sel_h, "ones": ones_h, "ident": ident_h,
            "pswap": pswap_h,
            "iotaC": iotaC_h, "iotaS": iotaS_h,
            "wg": wg_c, "wu": wu_c, "wd": wd_c,
            "swg": tile_gate(sg_c).astype(ml_bf16),
            "swu": tile_gate(su_c).astype(ml_bf16),
            "swd": sd_c.reshape(SIT, 128, D).astype(ml_bf16),
        })
    return in_maps


def kernel(**inputs) -> np.ndarray:
    if "nc" not in _CACHE:
        _CACHE["nc"] = _build()
    nc = _CACHE["nc"]
    in_maps = _prep_inputs(inputs)
    res = run_bass_kernel_spmd(nc, in_maps, core_ids=list(range(NCORES)), trace=TRACE)
    _CACHE["last_results"] = res
    out = np.concatenate([res.results[c]["out_sl"] for c in range(NCORES)], axis=0)
    return out.reshape(1, T, D).astype(np.float32)


# revision 27
# speedup vs baseline: 1.1067x; 1.0808x over previous
"""Trainium2 Bass kernel for an Aria-style MoE decoder layer (8-core SPMD).

Sharding:
  - Attention: head-parallel (20 heads -> 8 cores x 3 slots, 4 zero-padded),
    fp32r matmuls; o-projection partials combined with a fp32 ReduceScatter
    over the token axis (natural [T, D] layout).
  - Router/top-6: replicated per-token math on each core's 64-token slice,
    fp32; coefficients AllGathered.
  - Routed experts: expert-parallel, 2 experts/core, capacity 256/expert.
    Dispatch = one-hot gather matmul, combine = coefficient-weighted one-hot
    scatter matmul, all in bf16.
  - Shared expert: split along the intermediate dim (512 padded cols/core).
  - Final combine: bf16 ReduceScatter of MoE partials + local residual add.
"""

import numpy as np

import concourse.bass as bass
import concourse.mybir as mybir
import concourse.tile as tile
from concourse import bacc
from concourse.bass_utils import run_bass_kernel_spmd

try:
    import ml_dtypes
    ml_bf16 = ml_dtypes.bfloat16
except ImportError:  # pragma: no cover
    ml_bf16 = np.float16

F32 = mybir.dt.float32
F32R = mybir.dt.float32r
BF16 = mybir.dt.bfloat16
AF = mybir.ActivationFunctionType
ALU = mybir.AluOpType

NCORES = 8
T, D, NH, HD = 512, 2560, 20, 128
DT = D // 128            # 20 d-tiles
NSLOT = 3                # head slots per core (padded)
E, TOPK, EPC = 16, 6, 2  # experts, top-k, experts per core
I = 1664
IT = I // 128            # 13 i-tiles
C = 256                  # per-expert token capacity
SC = C // 128            # s-chunks per expert
SI = 512                 # shared-expert intermediate per core (416 padded)
SIT = SI // 128          # 4
TSL = T // NCORES        # 64 tokens per core slice
TCH = T // 128           # 4 token chunks
DCH = D // 512           # 5 d 512-chunks
EPS = 1e-6
ISQ = float(1.0 / np.sqrt(HD))

TRACE = False
_CACHE = {}


def _build():
    nc = bacc.Bacc("TRN2", target_bir_lowering=False, debug=False, num_devices=NCORES)

    def din(name, shape, dt):
        return nc.dram_tensor(name, shape, dt, kind="ExternalInput").ap()

    tn1T = din("tn1T", [128, DT, T], F32R)          # ln1-normed x, [dpart, dtile, tok]
    x_sl = din("x_sl", [TSL, D], F32)               # raw residual rows for this core
    wq = din("wq", [128, DT, NSLOT * 128], F32R)
    wk = din("wk", [128, DT, NSLOT * 128], F32R)
    wv = din("wv", [128, DT, NSLOT * 128], F32R)
    wo = din("wo", [128, NSLOT, D], F32R)
    cosT = din("cosT", [128, T], F32)
    sinT = din("sinT", [128, T], F32)               # rot-half signed sin, transposed
    maskT = din("maskT", [TCH, 128, T], F32)        # (mask & causal).T as [kc, krel, q]
    rw = din("rw", [128, DT, E], F32R)              # router weights (ln2 folded)
    sel = din("sel", [16, EPC], F32R)               # one-hot expert selector
    ones = din("ones", [128, 1], F32R)
    ident = din("ident", [128, 128], F32)
    pswap = din("pswap", [128, 128], F32R)          # rot-half permutation
    iotaC = din("iotaC", [128, C], F32)             # row p = [0..C-1]
    iotaS = din("iotaS", [128, SC], F32)            # col j = 128j + arange(128)
    wg = din("wg", [EPC, IT, 128, DT * 128], BF16)  # [e, it, dpart, (dtile,icol)]
    wu = din("wu", [EPC, IT, 128, DT * 128], BF16)
    wd = din("wd", [EPC, DCH, 128, IT * 512], BF16)  # [e, dc, ipart, (it,dcol)]
    swg = din("swg", [SIT, 128, DT * 128], BF16)    # [it, dpart, (dtile,icol)]
    swu = din("swu", [SIT, 128, DT * 128], BF16)
    swd = din("swd", [SIT, 128, D], BF16)           # [it, ipart, dcol]

    out_sl = nc.dram_tensor("out_sl", [TSL, D], F32, kind="ExternalOutput").ap()
    dbg_h = nc.dram_tensor("dbg_h", [TSL, D], F32, kind="ExternalOutput").ap()
    dbg_cw = nc.dram_tensor("dbg_cw", [T, E], F32, kind="ExternalOutput").ap()

    with tile.TileContext(nc) as tc:
      with tc.tile_pool(name="dram", bufs=1, space="DRAM") as dram, \
           tc.tile_pool(name="wpool", bufs=5) as dwp, \
           tc.tile_pool(name="consts", bufs=1) as cpool:
        rs1_in = dram.tile([T, D], F32, tag="rs1i")
        rs1_out = dram.tile([TSL, D], F32, tag="rs1o")
        ag_tn_in = dram.tile([TSL, D], BF16, tag="agti")
        ag_tn_out = dram.tile([T, D], BF16, tag="agto")
        ag_cw_in = dram.tile([TSL, E], F32, tag="agci")
        ag_cw_out = dram.tile([T, E], F32, tag="agco")
        rs2_in = dram.tile([T, D], BF16, tag="rs2i")
        rs2_out = dram.tile([TSL, D], BF16, tag="rs2o")

        ones_sb = cpool.tile([128, 1], F32R, tag="ones")
        id_sb = cpool.tile([128, 128], F32, tag="id")
        nc.sync.dma_start(ones_sb[:], ones[:])
        nc.sync.dma_start(id_sb[:], ident[:])

        # ================= Phase A: attention =================
        with nc.named_scope("attn"), \
             tc.tile_pool(name="a_big", bufs=1) as abig, \
             tc.tile_pool(name="a_w", bufs=6) as awp, \
             tc.tile_pool(name="a_wo", bufs=2) as awop, \
             tc.tile_pool(name="a_sb", bufs=1) as asb, \
             tc.tile_pool(name="a_tmp", bufs=2) as atmp:
            cos_sb = abig.tile([128, T], F32, tag="cos")
            sin_sb = abig.tile([128, T], F32, tag="sin")
            psw_sb = abig.tile([128, 128], F32R, tag="psw")
            nc.sync.dma_start(cos_sb[:], cosT[:])
            nc.sync.dma_start(sin_sb[:], sinT[:])
            nc.sync.dma_start(psw_sb[:], pswap[:])
            mask_sb = [abig.tile([128, T], F32, tag=f"mask{kc}", name=f"mask{kc}") for kc in range(TCH)]
            for kc in range(TCH):
                nc.sync.dma_start(mask_sb[kc][:], maskT[kc])
            tn1_sb = abig.tile([128, DT, T], F32R, tag="tn1")
            for dt_i in range(DT):
                nc.sync.dma_start(tn1_sb[:, dt_i, :], tn1T[:, dt_i, :])

            # --- Q, K (transposed layout [hd, tok]), with rope ---
            qk_out = []
            with tc.tile_pool(name="qk_ps", bufs=6, space="PSUM") as qkps, \
                 tc.tile_pool(name="sw_ps", bufs=2, space="PSUM") as swps:
                for which, w_ap in (("q", wq), ("k", wk)):
                    psums = [qkps.tile([128, T], F32, tag="qk", name=f"qk{which}{i}") for i in range(NSLOT)]
                    for dt_i in range(DT):
                        wt = awp.tile([128, NSLOT * 128], F32R, tag="wqkv",
                                      name=f"w{which}{dt_i}")
                        nc.sync.dma_start(wt[:], w_ap[:, dt_i, :])
                        for s in range(NSLOT):
                            nc.tensor.matmul(
                                psums[s][:], wt[:, s * 128:(s + 1) * 128],
                                tn1_sb[:, dt_i, :],
                                start=(dt_i == 0), stop=(dt_i == DT - 1))
                    outs = []
                    for s in range(NSLOT):
                        # rope: out = q*cos + swap64(q)*sin_signed.
                        # swap64 is a cross-partition move -> PE permutation.
                        qs = atmp.tile([128, T], F32R, tag="qs")
                        nc.vector.tensor_copy(qs[:], psums[s][:])
                        swp = swps.tile([128, T], F32, tag="swp")
                        nc.tensor.matmul(swp[:], psw_sb[:], qs[:],
                                         start=True, stop=True)
                        t1 = atmp.tile([128, T], F32, tag="t1")
                        nc.vector.tensor_mul(t1[:], qs[:], cos_sb[:])
                        t2 = atmp.tile([128, T], F32, tag="t2")
                        nc.vector.tensor_mul(t2[:], swp[:], sin_sb[:])
                        o = asb.tile([128, T], F32R, tag=f"rope{which}{s}")
                        nc.vector.tensor_add(o[:], t1[:], t2[:])
                        outs.append(o)
                    qk_out.append(outs)
            qT, kT = qk_out

            # --- V (natural layout [tok, slot*128]) ---
            v_sb = []
            with tc.tile_pool(name="v_ps", bufs=4, space="PSUM") as vps:
                vp_l = [vps.tile([128, NSLOT * 128], F32, tag="vps",
                                 name=f"vp{i}") for i in range(TCH)]
                for dt_i in range(DT):
                    wvt = awp.tile([128, NSLOT * 128], F32R, tag="wqkv",
                                   name=f"wv{dt_i}")
                    nc.sync.dma_start(wvt[:], wv[:, dt_i, :])
                    for tc_i in range(TCH):
                        nc.tensor.matmul(
                            vp_l[tc_i][:],
                            tn1_sb[:, dt_i, tc_i * 128:(tc_i + 1) * 128],
                            wvt[:], start=(dt_i == 0), stop=(dt_i == DT - 1))
                for tc_i in range(TCH):
                    vs = asb.tile([128, NSLOT * 128], F32R, tag=f"v{tc_i}",
                                  name=f"v{tc_i}")
                    nc.vector.tensor_copy(vs[:], vp_l[tc_i][:])
                    v_sb.append(vs)

            # --- scores -> exp -> mask -> AV + denom, per slot ---
            ctx_n = []
            with tc.tile_pool(name="s_ps", bufs=2, space="PSUM") as sps, \
                 tc.tile_pool(name="c_ps", bufs=2, space="PSUM") as ctps, \
                 tc.tile_pool(name="dn_ps", bufs=2, space="PSUM") as dnps:
                for s in range(NSLOT):
                    ctxp = ctps.tile([128, T], F32, tag="ctx")
                    denp = dnps.tile([1, T], F32, tag="den")
                    for kc in range(TCH):
                        ncols = T - kc * 128
                        q0 = kc * 128
                        sp = sps.tile([128, T], F32, tag="scores")
                        nc.tensor.matmul(
                            sp[:, 0:ncols], kT[s][:, q0:q0 + 128], qT[s][:, q0:T],
                            start=True, stop=True)
                        ex = atmp.tile([128, T], F32R, tag="exp")
                        nc.scalar.activation(ex[:, 0:ncols], sp[:, 0:ncols], AF.Exp,
                                             scale=ISQ)
                        nc.vector.tensor_mul(ex[:, 0:ncols], ex[:, 0:ncols],
                                             mask_sb[kc][:, q0:T])
                        nc.tensor.matmul(
                            ctxp[:, q0:T], v_sb[kc][:, s * 128:(s + 1) * 128],
                            ex[:, 0:ncols], start=(kc == 0), stop=(kc == TCH - 1))
                        nc.tensor.matmul(
                            denp[:, q0:T], ones_sb[:], ex[:, 0:ncols],
                            start=(kc == 0), stop=(kc == TCH - 1))
                    rec = atmp.tile([1, T], F32, tag="rec")
                    nc.vector.reciprocal(rec[:], denp[:])
                    bc = atmp.tile([128, T], F32, tag="bc")
                    nc.gpsimd.partition_broadcast(bc[:], rec[:])
                    cn = asb.tile([128, T], F32R, tag=f"ctxn{s}")
                    nc.vector.tensor_mul(cn[:], ctxp[:], bc[:])
                    ctx_n.append(cn)

            # --- o-projection, natural [tok, d] output -> rs1_in (fp32) ---
            with tc.tile_pool(name="o_ps", bufs=2, space="PSUM") as ops_p:
                for dc in range(DCH):
                    wot = awop.tile([128, NSLOT, 512], F32R, tag="wo",
                                   name=f"wo{dc}")
                    nc.sync.dma_start(wot[:], wo[:, :, dc * 512:(dc + 1) * 512])
                    for tc_i in range(TCH):
                        op = ops_p.tile([128, 512], F32, tag="ops")
                        for s in range(NSLOT):
                            nc.tensor.matmul(
                                op[:], ctx_n[s][:, tc_i * 128:(tc_i + 1) * 128],
                                wot[:, s, :],
                                start=(s == 0), stop=(s == NSLOT - 1))
                        ob = atmp.tile([128, 512], F32, tag="ob")
                        nc.vector.tensor_copy(ob[:], op[:])
                        nc.sync.dma_start(
                            rs1_in[:].rearrange("(c p) d -> c p d", p=128)[tc_i, :, dc * 512:(dc + 1) * 512],
                            ob[:])

        nc.gpsimd.collective_compute(
            "ReduceScatter", ALU.add, replica_groups=[list(range(NCORES))],
            ins=[rs1_in.opt()], outs=[rs1_out.opt()])

        hpool_cm = tc.tile_pool(name="hpool", bufs=1)
        hpool = hpool_cm.__enter__()
        mpool_cm = tc.tile_pool(name="mpool", bufs=1)
        mpool = mpool_cm.__enter__()
        h_nat = hpool.tile([TSL, D], F32, tag="h")

        # ================= Phase B: h, ln2, router, top-k =================
        with nc.named_scope("router"), \
             tc.tile_pool(name="bwork", bufs=1) as bw, \
             tc.tile_pool(name="b_ps", bufs=2, space="PSUM") as bps, \
             tc.tile_pool(name="b_ps1", bufs=1, space="PSUM") as bps1:
            o_sl = bw.tile([TSL, D], F32, tag="osl")
            nc.sync.dma_start(o_sl[:], rs1_out[:])
            x_sb = bw.tile([TSL, D], F32, tag="xsl")
            nc.sync.dma_start(x_sb[:], x_sl[:])
            nc.vector.tensor_add(h_nat[:], x_sb[:], o_sl[:])
            nc.sync.dma_start(dbg_h[:], h_nat[:])

            sq = bw.tile([TSL, D], F32, tag="sq")
            ssq = bw.tile([TSL, 1], F32, tag="ssq")
            nc.scalar.activation(sq[:], h_nat[:], AF.Square, accum_out=ssq[:])
            eps_t = bw.tile([TSL, 1], F32, tag="epst")
            nc.gpsimd.memset(eps_t[:], EPS)
            rms = bw.tile([TSL, 1], F32, tag="rms")
            nc.scalar.activation(rms[:], ssq[:], AF.Sqrt, scale=float(1.0 / D),
                                 bias=eps_t[:])
            inv = bw.tile([TSL, 1], F32, tag="inv")
            nc.vector.reciprocal(inv[:], rms[:])
            t_sl = bw.tile([TSL, D], F32, tag="tsl")
            nc.vector.tensor_scalar_mul(t_sl[:], h_nat[:], inv[:])

            # bf16 copy of normalized slice -> AllGather input
            t_sl_bf = bw.tile([TSL, D], BF16, tag="tslbf")
            nc.vector.tensor_copy(t_sl_bf[:], t_sl[:])
            nc.sync.dma_start(ag_tn_in[:], t_sl_bf[:])

            # transpose slice -> tT_cols [d, tok] f32r for the router matmul
            rwt = bw.tile([128, DT, E], F32R, tag="rw")
            nc.sync.dma_start(rwt[:], rw[:])
            tts_l = []
            for dt_i in range(DT):
                tp = bps.tile([128, TSL], F32, tag="ttp")
                nc.tensor.transpose(tp[:], t_sl[:, dt_i * 128:(dt_i + 1) * 128],
                                    id_sb[0:TSL, 0:TSL])
                tts = bw.tile([128, TSL], F32R, tag=f"tts{dt_i}")
                nc.vector.tensor_copy(tts[:], tp[:])
                tts_l.append(tts)
            lg = bps1.tile([TSL, E], F32, tag="lg")
            for dt_i in range(DT):
                nc.tensor.matmul(lg[:], tts_l[dt_i][:], rwt[:, dt_i, :],
                                 start=(dt_i == 0), stop=(dt_i == DT - 1))

            # top-k on unnormalized exp(logits): same selection and, since
            # cw = top6 / sum(top6), the softmax denominator cancels exactly.
            ex = bw.tile([TSL, E], F32, tag="exr")
            nc.scalar.activation(ex[:], lg[:], AF.Exp)
            probs = ex

            # iterative top-6: extract max 6 times
            work = bw.tile([TSL, E], F32, tag="work")
            nc.vector.tensor_copy(work[:], probs[:])
            tsum = bw.tile([TSL, 1], F32, tag="tsum")
            thr = bw.tile([TSL, 1], F32, tag="thr")
            for k in range(TOPK):
                m = bw.tile([TSL, 1], F32, tag=f"m{k}")
                nc.vector.tensor_reduce(m[:], work[:], mybir.AxisListType.X, ALU.max)
                if k == 0:
                    nc.vector.tensor_copy(tsum[:], m[:])
                else:
                    nc.vector.tensor_add(tsum[:], tsum[:], m[:])
                if k == TOPK - 1:
                    nc.vector.tensor_copy(thr[:], m[:])
                else:
                    eq = bw.tile([TSL, E], F32, tag="eq")
                    nc.vector.tensor_scalar(eq[:], work[:], m[:], 1e9,
                                            op0=ALU.is_ge, op1=ALU.mult)
                    nc.vector.tensor_sub(work[:], work[:], eq[:])
            mask6 = bw.tile([TSL, E], F32, tag="mask6")
            nc.vector.tensor_scalar(mask6[:], probs[:], thr[:], None, op0=ALU.is_ge)
            cwu = bw.tile([TSL, E], F32, tag="cwu")
            nc.vector.tensor_mul(cwu[:], probs[:], mask6[:])
            rts = bw.tile([TSL, 1], F32, tag="rts")
            nc.vector.reciprocal(rts[:], tsum[:])
            cw = bw.tile([TSL, E], F32, tag="cw")
            nc.vector.tensor_scalar_mul(cw[:], cwu[:], rts[:])
            nc.sync.dma_start(ag_cw_in[:], cw[:])

        nc.gpsimd.collective_compute(
            "AllGather", ALU.bypass, replica_groups=[list(range(NCORES))],
            ins=[ag_cw_in.opt()], outs=[ag_cw_out.opt()])
        nc.gpsimd.collective_compute(
            "AllGather", ALU.bypass, replica_groups=[list(range(NCORES))],
            ins=[ag_tn_in.opt()], outs=[ag_tn_out.opt()])
        nc.sync.dma_start(dbg_cw[:], ag_cw_out[:])

        # ================= Phase C: dispatch prep =================
        # mpool holds what phase D needs: tT, tcT, STw, (later act/down tiles)
        tT = [mpool.tile([128, T], BF16, tag=f"tT{i}", name=f"tT{i}") for i in range(DT)]
        STw = {}
        tcT = {}
        with nc.named_scope("dispatch"), \
             tc.tile_pool(name="c_sb", bufs=1) as csb, \
             tc.tile_pool(name="c_tmp", bufs=1) as ctmp:
            tnat = []
            for tc_i in range(TCH):
                tt = csb.tile([128, D], BF16, tag=f"tnat{tc_i}")
                nc.sync.dma_start(
                    tt[:], ag_tn_out[:].rearrange("(c p) d -> c p d", p=128)[tc_i])
                tnat.append(tt)
            idb = csb.tile([128, 128], BF16, tag="idb")
            nc.vector.tensor_copy(idb[:], id_sb[:])
            with tc.tile_pool(name="ct_ps", bufs=4, space="PSUM") as ctp:
                for dt_i in range(DT):
                    for tc_i in range(TCH):
                        tp = ctp.tile([128, 128], BF16, tag="ttp")
                        nc.tensor.matmul(tp[:],
                                         tnat[tc_i][:, dt_i * 128:(dt_i + 1) * 128],
                                         idb[:], is_transpose=True,
                                         start=True, stop=True)
                        nc.vector.tensor_copy(
                            tT[dt_i][:, tc_i * 128:(tc_i + 1) * 128], tp[:])

            # cw -> cwT -> my 2 experts' rows; mask/pos/posm
            with tc.tile_pool(name="cw_ps", bufs=1, space="PSUM") as cwps, \
                 tc.tile_pool(name="cw_ps2", bufs=2, space="PSUM") as cwps2:
                cwn = []
                for tc_i in range(TCH):
                    cn = ctmp.tile([128, E], F32, tag="cwn")
                    nc.sync.dma_start(
                        cn[:], ag_cw_out[:].rearrange("(c p) e -> c p e", p=128)[tc_i])
                    cwn.append(cn)
                cwTp = cwps.tile([E, T], F32, tag="cwT")
                for tc_i in range(TCH):
                    nc.tensor.matmul(cwTp[:, tc_i * 128:(tc_i + 1) * 128],
                                     cwn[tc_i][:], id_sb[:], is_transpose=True,
                                     start=True, stop=True)
                cwT = csb.tile([E, T], F32R, tag="cwTs")
                nc.vector.tensor_copy(cwT[:], cwTp[:])
                selt = csb.tile([16, EPC], F32R, tag="sel")
                nc.sync.dma_start(selt[:], sel[:])
                zz = csb.tile([1, T], F32, tag="zz")
                nc.gpsimd.memset(zz[:], 0.0)
                pn = [csb.tile([128, EPC], F32, tag=f"pn{i}", name=f"pn{i}")
                      for i in range(TCH)]
                cwm = []
                posm = []
                for e in range(EPC):
                    # this expert's coefficient row, at partition 0
                    cwmp = cwps.tile([1, T], F32, tag=f"cwm{e}", name=f"cwmp{e}")
                    nc.tensor.matmul(cwmp[:], selt[:, e:e + 1], cwT[:],
                                     start=True, stop=True)
                    cwm_e = csb.tile([1, T], F32, tag=f"cwms{e}", name=f"cwms{e}")
                    nc.vector.tensor_copy(cwm_e[:], cwmp[:])
                    cwm.append(cwm_e)
                    msk = csb.tile([1, T], F32, tag=f"msk{e}", name=f"msk{e}")
                    nc.vector.tensor_scalar(msk[:], cwm_e[:], 0.0, None,
                                            op0=ALU.is_gt)
                    pos = csb.tile([1, T], F32, tag=f"pos{e}", name=f"pos{e}")
                    nc.vector.tensor_tensor_scan(pos[:], msk[:], zz[:], 0.0,
                                                 op0=ALU.add, op1=ALU.add)
                    pm = csb.tile([1, T], F32, tag=f"posm{e}", name=f"posm{e}")
                    nc.vector.tensor_mul(pm[:], pos[:], msk[:])
                    nc.vector.tensor_scalar_add(pm[:], pm[:], -1.0)
                    posm.append(pm)
                    # posm_nat [tok, 1] per token chunk -> pn[tc][:, e]
                    for tc_i in range(TCH):
                        pp = cwps2.tile([128, 1], F32, tag="pn")
                        nc.tensor.matmul(pp[:],
                                         pm[:, tc_i * 128:(tc_i + 1) * 128],
                                         id_sb[0:1, 0:1], is_transpose=True,
                                         start=True, stop=True)
                        nc.vector.tensor_copy(pn[tc_i][:, e:e + 1], pp[:])

            # S (gather) and cw-weighted ST (scatter) one-hot matrices
            iC = csb.tile([128, C], F32, tag="iC")
            nc.sync.dma_start(iC[:], iotaC[:])
            iS = csb.tile([128, SC], F32, tag="iS")
            nc.sync.dma_start(iS[:], iotaS[:])
            S = {}
            for e in range(EPC):
                for tc_i in range(TCH):
                    st = csb.tile([128, C], BF16, tag=f"S{e}_{tc_i}")
                    nc.vector.tensor_scalar(st[:], iC[:], pn[tc_i][:, e:e + 1], None,
                                            op0=ALU.is_equal)
                    S[(e, tc_i)] = st
                pb = ctmp.tile([128, T], F32, tag="pb")
                nc.gpsimd.partition_broadcast(pb[:], posm[e][:])
                cb = ctmp.tile([128, T], F32, tag="cb")
                nc.gpsimd.partition_broadcast(cb[:], cwm[e][:])
                for sc_i in range(SC):
                    t1 = ctmp.tile([128, T], F32, tag="st1")
                    nc.vector.tensor_scalar(t1[:], pb[:], iS[:, sc_i:sc_i + 1], None,
                                            op0=ALU.is_equal)
                    stw = mpool.tile([128, T], BF16, tag=f"STw{e}_{sc_i}")
                    nc.vector.tensor_mul(stw[:], t1[:], cb[:])
                    STw[(e, sc_i)] = stw

            # gather: tcT[e] tiles [128(d), C] bf16
            with tc.tile_pool(name="g_ps", bufs=3, space="PSUM") as gps_p:
                for e in range(EPC):
                    for dt_i in range(DT):
                        gp = gps_p.tile([128, C], F32, tag="gps")
                        for tc_i in range(TCH):
                            nc.tensor.matmul(
                                gp[:], tnat[tc_i][:, dt_i * 128:(dt_i + 1) * 128],
                                S[(e, tc_i)][:],
                                start=(tc_i == 0), stop=(tc_i == TCH - 1))
                        g = mpool.tile([128, C], BF16, tag=f"tcT{e}_{dt_i}")
                        nc.vector.tensor_copy(g[:], gp[:])
                        tcT[(e, dt_i)] = g

        # ================= Phase D: experts =================
        act = {}
        act_sh = []
        down = {}
        with nc.named_scope("experts"), \
             tc.tile_pool(name="d_wd", bufs=2) as dwdp, \
             tc.tile_pool(name="d_sb", bufs=2) as dsb:
            with tc.tile_pool(name="gu_ps", bufs=2, space="PSUM") as gups:
                # routed gate/up -> act (bf16, kept in mpool until down)
                for e in range(EPC):
                    for it in range(IT):
                        wgt = dwp.tile([128, DT * 128], BF16, tag="wg")
                        nc.gpsimd.dma_start(wgt[:], wg[e, it])
                        gp = gups.tile([128, T], F32, tag="gps")
                        for dt_i in range(DT):
                            nc.tensor.matmul(gp[:, 0:C],
                                             wgt[:, dt_i * 128:(dt_i + 1) * 128],
                                             tcT[(e, dt_i)][:],
                                             start=(dt_i == 0), stop=(dt_i == DT - 1))
                        wut = dwp.tile([128, DT * 128], BF16, tag="wu")
                        nc.gpsimd.dma_start(wut[:], wu[e, it])
                        up = gups.tile([128, T], F32, tag="ups")
                        for dt_i in range(DT):
                            nc.tensor.matmul(up[:, 0:C],
                                             wut[:, dt_i * 128:(dt_i + 1) * 128],
                                             tcT[(e, dt_i)][:],
                                             start=(dt_i == 0), stop=(dt_i == DT - 1))
                        sl = dsb.tile([128, C], F32, tag="sl")
                        nc.scalar.activation(sl[:], gp[:, 0:C], AF.Silu)
                        a = mpool.tile([128, C], BF16, tag=f"act{e}_{it}")
                        nc.vector.tensor_mul(a[:], sl[:], up[:, 0:C])
                        act[(e, it)] = a

                # shared expert gate/up -> act_sh
                for it in range(SIT):
                    sgt = dwp.tile([128, DT * 128], BF16, tag="wg")
                    nc.gpsimd.dma_start(sgt[:], swg[it])
                    gp = gups.tile([128, T], F32, tag="gps")
                    for dt_i in range(DT):
                        nc.tensor.matmul(gp[:], sgt[:, dt_i * 128:(dt_i + 1) * 128],
                                         tT[dt_i][:],
                                         start=(dt_i == 0), stop=(dt_i == DT - 1))
                    sut = dwp.tile([128, DT * 128], BF16, tag="wu")
                    nc.gpsimd.dma_start(sut[:], swu[it])
                    up = gups.tile([128, T], F32, tag="ups")
                    for dt_i in range(DT):
                        nc.tensor.matmul(up[:], sut[:, dt_i * 128:(dt_i + 1) * 128],
                                         tT[dt_i][:],
                                         start=(dt_i == 0), stop=(dt_i == DT - 1))
                    sl = dsb.tile([128, T], F32, tag="ssl")
                    nc.scalar.activation(sl[:], gp[:], AF.Silu)
                    a = mpool.tile([128, T], BF16, tag=f"acts{it}")
                    nc.vector.tensor_mul(a[:], sl[:], up[:])
                    act_sh.append(a)

            # routed down: [s, d] = act.T @ wd, accumulated over i-tiles
            with tc.tile_pool(name="dn_ps", bufs=4, space="PSUM") as dnps:
                for e in range(EPC):
                    for dc in range(DCH):
                        wdt = dwdp.tile([128, IT * 512], BF16, tag="wd")
                        nc.gpsimd.dma_start(wdt[:], wd[e, dc])
                        for sc_i in range(SC):
                            dp = dnps.tile([128, 512], F32, tag="dn")
                            for it in range(IT):
                                nc.tensor.matmul(
                                    dp[:],
                                    act[(e, it)][:, sc_i * 128:(sc_i + 1) * 128],
                                    wdt[:, it * 512:(it + 1) * 512],
                                    start=(it == 0), stop=(it == IT - 1))
                            db = mpool.tile([128, 512], BF16, tag=f"db{e}_{sc_i}_{dc}")
                            nc.vector.tensor_copy(db[:], dp[:])
                            down[(e, sc_i, dc)] = db

        # final: scatter routed + shared down -> rs2_in [T, D] bf16
        with nc.named_scope("combine"), \
             tc.tile_pool(name="f_w", bufs=1) as fwp, \
             tc.tile_pool(name="f_sb", bufs=3) as fsb, \
             tc.tile_pool(name="f_ps", bufs=3, space="PSUM") as fps_p:
            swd_sb = []
            for it in range(SIT):
                sdt = fwp.tile([128, D], BF16, tag=f"swd{it}")
                nc.gpsimd.dma_start(sdt[:], swd[it])
                swd_sb.append(sdt)
            for tc_i in range(TCH):
                for dc in range(DCH):
                    fp = fps_p.tile([128, 512], F32, tag="fps")
                    for it in range(SIT):
                        nc.tensor.matmul(
                            fp[:], act_sh[it][:, tc_i * 128:(tc_i + 1) * 128],
                            swd_sb[it][:, dc * 512:(dc + 1) * 512],
                            start=(it == 0), stop=False)
                    n_sc = EPC * SC
                    cnt = 0
                    for e in range(EPC):
                        for sc_i in range(SC):
                            cnt += 1
                            nc.tensor.matmul(
                                fp[:],
                                STw[(e, sc_i)][:, tc_i * 128:(tc_i + 1) * 128],
                                down[(e, sc_i, dc)][:],
                                start=False, stop=(cnt == n_sc))
                    fb = fsb.tile([128, 512], BF16, tag="fb")
                    nc.vector.tensor_copy(fb[:], fp[:])
                    nc.sync.dma_start(
                        rs2_in[:].rearrange("(c p) d -> c p d", p=128)[tc_i, :, dc * 512:(dc + 1) * 512],
                        fb[:])

        nc.gpsimd.collective_compute(
            "ReduceScatter", ALU.add, replica_groups=[list(range(NCORES))],
            ins=[rs2_in.opt()], outs=[rs2_out.opt()])

        # ================= Final: residual add =================
        with tc.tile_pool(name="fin", bufs=1) as fin:
            moe_bf = fin.tile([TSL, D], BF16, tag="moebf")
            nc.sync.dma_start(moe_bf[:], rs2_out[:])
            moe_f = fin.tile([TSL, D], F32, tag="moef")
            nc.vector.tensor_copy(moe_f[:], moe_bf[:])
            o = fin.tile([TSL, D], F32, tag="o")
            nc.vector.tensor_add(o[:], h_nat[:], moe_f[:])
            nc.sync.dma_start(out_sl[:], o[:])

        mpool_cm.__exit__(None, None, None)
        hpool_cm.__exit__(None, None, None)

    nc.compile()
    return nc


def _prep_inputs(inputs):
    """Host-side prep: norms/tables/layout/sharding. Returns in_maps[8]."""
    f32 = np.float32
    x = np.asarray(inputs["hidden_states"], dtype=f32).reshape(T, D)
    ln1w = np.asarray(inputs["ln1_w"], dtype=f32)
    ln2w = np.asarray(inputs["ln2_w"], dtype=f32)
    pos_ids = np.asarray(inputs["position_ids"]).reshape(T)
    amask = np.asarray(inputs["attention_mask"]).reshape(T, T)

    xd = x.astype(np.float64)
    inv1 = 1.0 / np.sqrt((xd * xd).mean(axis=1, keepdims=True) + EPS)
    tn1 = ((xd * inv1).astype(f32)) * ln1w[None, :]
    tn1T_t = np.ascontiguousarray(
        tn1.T.reshape(DT, 128, T).transpose(1, 0, 2)).astype(f32)

    inv_freq = 1.0 / (5e6 ** (np.arange(0, HD, 2, dtype=f32) / HD))
    ang = pos_ids.astype(f32)[:, None] * inv_freq[None, :]        # [T, 64]
    cos_f = np.concatenate([np.cos(ang), np.cos(ang)], axis=1)    # [T, 128]
    sin_h = np.sin(ang)
    sinT_h = np.concatenate([-sin_h, sin_h], axis=1).T            # [128, T]
    cosT_h = np.ascontiguousarray(cos_f.T).astype(f32)
    sinT_h = np.ascontiguousarray(sinT_h).astype(f32)

    causal = np.tril(np.ones((T, T), dtype=bool))
    mk = (amask & causal).T.astype(f32)                           # [k, q]
    maskT_h = np.ascontiguousarray(mk.reshape(TCH, 128, T)).astype(f32)

    wq_f = np.asarray(inputs["wq"], dtype=f32)
    wk_f = np.asarray(inputs["wk"], dtype=f32)
    wv_f = np.asarray(inputs["wv"], dtype=f32)
    wo_f = np.asarray(inputs["wo"], dtype=f32)
    rw_f = np.asarray(inputs["router_w"], dtype=f32) * ln2w[:, None]
    wg_f = np.asarray(inputs["w_gate"], dtype=f32) * ln2w[None, :, None]
    wu_f = np.asarray(inputs["w_up"], dtype=f32) * ln2w[None, :, None]
    wd_f = np.asarray(inputs["w_down"], dtype=f32)
    sg_f = np.asarray(inputs["sw_gate"], dtype=f32) * ln2w[:, None]
    su_f = np.asarray(inputs["sw_up"], dtype=f32) * ln2w[:, None]
    sd_f = np.asarray(inputs["sw_down"], dtype=f32)

    rw_t = np.ascontiguousarray(rw_f.reshape(DT, 128, E).transpose(1, 0, 2))

    head_of = []
    for c in range(NCORES):
        if c < 4:
            head_of.append([3 * c, 3 * c + 1, 3 * c + 2])
        else:
            head_of.append([12 + 2 * (c - 4), 13 + 2 * (c - 4), None])

    iotaC_h = np.broadcast_to(np.arange(C, dtype=f32), (128, C)).copy()
    iotaS_h = (np.arange(128, dtype=f32)[:, None]
               + 128.0 * np.arange(SC, dtype=f32)[None, :]).copy()
    ident_h = np.eye(128, dtype=f32)
    ones_h = np.ones((128, 1), dtype=f32)
    # pswap[d, j] = 1 iff d == (j + 64) % 128, so (P.T @ q)[j] = q[(j+64)%128]
    pswap_h = np.zeros((128, 128), dtype=f32)
    for j in range(128):
        pswap_h[(j + 64) % 128, j] = 1.0

    sip = 3328 // NCORES  # 416

    def tile_gate(w):  # [D, SI] -> [SIT, 128, DT*128]
        return np.ascontiguousarray(
            w.reshape(DT, 128, SIT, 128).transpose(2, 1, 0, 3).reshape(
                SIT, 128, DT * 128))

    def exp_gate(w2):  # [D, I] -> [IT, 128, DT*128]
        return np.ascontiguousarray(
            w2.reshape(DT, 128, IT, 128).transpose(2, 1, 0, 3).reshape(
                IT, 128, DT * 128))

    def exp_down(w2):  # [I, D] -> [DCH, 128, IT*512]
        return np.ascontiguousarray(
            w2.reshape(IT, 128, DCH, 512).transpose(2, 1, 0, 3).reshape(
                DCH, 128, IT * 512))

    in_maps = []
    for c in range(NCORES):
        hs = head_of[c]
        wq_c = np.zeros((D, NSLOT * 128), f32)
        wk_c = np.zeros((D, NSLOT * 128), f32)
        wv_c = np.zeros((D, NSLOT * 128), f32)
        wo_c = np.zeros((NSLOT * 128, D), f32)
        for s, h in enumerate(hs):
            if h is None:
                continue
            wq_c[:, s * 128:(s + 1) * 128] = wq_f[:, h * 128:(h + 1) * 128]
            wk_c[:, s * 128:(s + 1) * 128] = wk_f[:, h * 128:(h + 1) * 128]
            wv_c[:, s * 128:(s + 1) * 128] = wv_f[:, h * 128:(h + 1) * 128]
            wo_c[s * 128:(s + 1) * 128, :] = wo_f[h * 128:(h + 1) * 128, :]

        def qt(w):  # [D, 384] -> [128, DT, 384]
            return np.ascontiguousarray(
                w.reshape(DT, 128, NSLOT * 128).transpose(1, 0, 2))

        wo_t = np.ascontiguousarray(wo_c.reshape(NSLOT, 128, D).transpose(1, 0, 2))

        e0, e1 = 2 * c, 2 * c + 1
        sel_h = np.zeros((16, EPC), f32)
        sel_h[e0, 0] = 1.0
        sel_h[e1, 1] = 1.0

        wg_c = np.stack([exp_gate(wg_f[e0]), exp_gate(wg_f[e1])]).astype(ml_bf16)
        wu_c = np.stack([exp_gate(wu_f[e0]), exp_gate(wu_f[e1])]).astype(ml_bf16)
        wd_c = np.stack([exp_down(wd_f[e0]), exp_down(wd_f[e1])]).astype(ml_bf16)

        s0 = c * sip
        sg_c = np.zeros((D, SI), f32)
        su_c = np.zeros((D, SI), f32)
        sd_c = np.zeros((SI, D), f32)
        sg_c[:, :sip] = sg_f[:, s0:s0 + sip]
        su_c[:, :sip] = su_f[:, s0:s0 + sip]
        sd_c[:sip, :] = sd_f[s0:s0 + sip, :]

        in_maps.append({
            "tn1T": tn1T_t,
            "x_sl": np.ascontiguousarray(x[c * TSL:(c + 1) * TSL]),
            "wq": qt(wq_c), "wk": qt(wk_c), "wv": qt(wv_c), "wo": wo_t,
            "cosT": cosT_h, "sinT": sinT_h, "maskT": maskT_h,
            "rw": rw_t, "sel": sel_h, "ones": ones_h, "ident": ident_h,
            "pswap": pswap_h,
            "iotaC": iotaC_h, "iotaS": iotaS_h,
            "wg": wg_c, "wu": wu_c, "wd": wd_c,
            "swg": tile_gate(sg_c).astype(ml_bf16),
            "swu": tile_gate(su_c).astype(ml_bf16),
            "swd": sd_c.reshape(SIT, 128, D).astype(ml_bf16),
        })
    return in_maps


def kernel(**inputs) -> np.ndarray:
    if "nc" not in _CACHE:
        _CACHE["nc"] = _build()
    nc = _CACHE["nc"]
    in_maps = _prep_inputs(inputs)
    res = run_bass_kernel_spmd(nc, in_maps, core_ids=list(range(NCORES)), trace=TRACE)
    _CACHE["last_results"] = res
    out = np.concatenate([res.results[c]["out_sl"] for c in range(NCORES)], axis=0)
    return out.reshape(1, T, D).astype(np.float32)


# revision 29
# speedup vs baseline: 1.1339x; 1.0246x over previous
"""Trainium2 Bass kernel for an Aria-style MoE decoder layer (8-core SPMD).

Sharding:
  - Attention: head-parallel (20 heads -> 8 cores x 3 slots, 4 zero-padded),
    fp32r matmuls; o-projection partials combined with a fp32 ReduceScatter
    over the token axis (natural [T, D] layout).
  - Router/top-6: replicated per-token math on each core's 64-token slice,
    fp32; coefficients AllGathered.
  - Routed experts: expert-parallel, 2 experts/core, capacity 256/expert.
    Dispatch = one-hot gather matmul, combine = coefficient-weighted one-hot
    scatter matmul, all in bf16.
  - Shared expert: split along the intermediate dim (512 padded cols/core).
  - Final combine: bf16 ReduceScatter of MoE partials + local residual add.
"""

import numpy as np

import concourse.bass as bass
import concourse.mybir as mybir
import concourse.tile as tile
from concourse import bacc
from concourse.bass_utils import run_bass_kernel_spmd

try:
    import ml_dtypes
    ml_bf16 = ml_dtypes.bfloat16
except ImportError:  # pragma: no cover
    ml_bf16 = np.float16

F32 = mybir.dt.float32
F32R = mybir.dt.float32r
BF16 = mybir.dt.bfloat16
AF = mybir.ActivationFunctionType
ALU = mybir.AluOpType

NCORES = 8
T, D, NH, HD = 512, 2560, 20, 128
DT = D // 128            # 20 d-tiles
NSLOT = 3                # head slots per core (padded)
E, TOPK, EPC = 16, 6, 2  # experts, top-k, experts per core
I = 1664
IT = I // 128            # 13 i-tiles
C = 256                  # per-expert token capacity
SC = C // 128            # s-chunks per expert
SI = 512                 # shared-expert intermediate per core (416 padded)
SIT = SI // 128          # 4
TSL = T // NCORES        # 64 tokens per core slice
TCH = T // 128           # 4 token chunks
DCH = D // 512           # 5 d 512-chunks
EPS = 1e-6
ISQ = float(1.0 / np.sqrt(HD))

TRACE = False
_CACHE = {}


def _build():
    nc = bacc.Bacc("TRN2", target_bir_lowering=False, debug=False, num_devices=NCORES)

    def din(name, shape, dt):
        return nc.dram_tensor(name, shape, dt, kind="ExternalInput").ap()

    tn1T = din("tn1T", [128, DT, T], F32R)          # ln1-normed x, [dpart, dtile, tok]
    x_sl = din("x_sl", [TSL, D], F32)               # raw residual rows for this core
    wq = din("wq", [128, DT, NSLOT * 128], F32R)
    wk = din("wk", [128, DT, NSLOT * 128], F32R)
    wv = din("wv", [128, DT, NSLOT * 128], F32R)
    wo = din("wo", [128, NSLOT, D], F32R)
    cosT = din("cosT", [128, T], F32)
    sinT = din("sinT", [128, T], F32)               # rot-half signed sin, transposed
    maskT = din("maskT", [TCH, 128, T], F32)        # (mask & causal).T as [kc, krel, q]
    rw = din("rw", [128, DT, E], F32R)              # router weights (ln2 folded)
    sel = din("sel", [16, EPC], F32R)               # one-hot expert selector
    ones = din("ones", [128, 1], F32R)
    ident = din("ident", [128, 128], F32)
    pswap = din("pswap", [128, 128], F32R)          # rot-half permutation
    iotaC = din("iotaC", [128, C], F32)             # row p = [0..C-1]
    iotaS = din("iotaS", [128, SC], F32)            # col j = 128j + arange(128)
    wg = din("wg", [EPC, IT, 128, DT * 128], BF16)  # [e, it, dpart, (dtile,icol)]
    wu = din("wu", [EPC, IT, 128, DT * 128], BF16)
    wd = din("wd", [EPC, DCH, 128, IT * 512], BF16)  # [e, dc, ipart, (it,dcol)]
    swg = din("swg", [SIT, 128, DT * 128], BF16)    # [it, dpart, (dtile,icol)]
    swu = din("swu", [SIT, 128, DT * 128], BF16)
    swd = din("swd", [SIT, 128, D], BF16)           # [it, ipart, dcol]

    out_sl = nc.dram_tensor("out_sl", [TSL, D], F32, kind="ExternalOutput").ap()
    dbg_h = nc.dram_tensor("dbg_h", [TSL, D], F32, kind="ExternalOutput").ap()
    dbg_cw = nc.dram_tensor("dbg_cw", [T, E], F32, kind="ExternalOutput").ap()

    with tile.TileContext(nc) as tc:
      with tc.tile_pool(name="dram", bufs=1, space="DRAM") as dram, \
           tc.tile_pool(name="wpool", bufs=4) as dwp, \
           tc.tile_pool(name="consts", bufs=1) as cpool:
        rs1_in_a = dram.tile([T, 1536], F32, tag="rs1ia")
        rs1_in_b = dram.tile([T, 1024], F32, tag="rs1ib")
        rs1_out_a = dram.tile([TSL, 1536], F32, tag="rs1oa")
        rs1_out_b = dram.tile([TSL, 1024], F32, tag="rs1ob")
        ag_tn_in = dram.tile([TSL, D], BF16, tag="agti")
        ag_tn_out = dram.tile([T, D], BF16, tag="agto")
        ag_cw_in = dram.tile([TSL, E], F32, tag="agci")
        ag_cw_out = dram.tile([T, E], F32, tag="agco")
        rs2_in = dram.tile([T, D], BF16, tag="rs2i")
        rs2_out = dram.tile([TSL, D], BF16, tag="rs2o")

        ones_sb = cpool.tile([128, 1], F32R, tag="ones")
        id_sb = cpool.tile([128, 128], F32, tag="id")
        nc.sync.dma_start(ones_sb[:], ones[:])
        nc.sync.dma_start(id_sb[:], ident[:])

        # ================= Phase A: attention =================
        with nc.named_scope("attn"), \
             tc.tile_pool(name="a_big", bufs=1) as abig, \
             tc.tile_pool(name="a_w", bufs=6) as awp, \
             tc.tile_pool(name="a_wo", bufs=2) as awop, \
             tc.tile_pool(name="a_sb", bufs=1) as asb, \
             tc.tile_pool(name="a_tmp", bufs=2) as atmp:
            cos_sb = abig.tile([128, T], F32, tag="cos")
            sin_sb = abig.tile([128, T], F32, tag="sin")
            psw_sb = abig.tile([128, 128], F32R, tag="psw")
            nc.sync.dma_start(cos_sb[:], cosT[:])
            nc.sync.dma_start(sin_sb[:], sinT[:])
            nc.sync.dma_start(psw_sb[:], pswap[:])
            mask_sb = [abig.tile([128, T], F32, tag=f"mask{kc}", name=f"mask{kc}") for kc in range(TCH)]
            for kc in range(TCH):
                nc.sync.dma_start(mask_sb[kc][:], maskT[kc])
            tn1_sb = abig.tile([128, DT, T], F32R, tag="tn1")
            for dt_i in range(DT):
                nc.sync.dma_start(tn1_sb[:, dt_i, :], tn1T[:, dt_i, :])

            # --- Q, K (transposed layout [hd, tok]), with rope ---
            qk_out = []
            with tc.tile_pool(name="qk_ps", bufs=6, space="PSUM") as qkps, \
                 tc.tile_pool(name="sw_ps", bufs=2, space="PSUM") as swps:
                for which, w_ap in (("q", wq), ("k", wk)):
                    psums = [qkps.tile([128, T], F32, tag="qk", name=f"qk{which}{i}") for i in range(NSLOT)]
                    for dt_i in range(DT):
                        wt = awp.tile([128, NSLOT * 128], F32R, tag="wqkv",
                                      name=f"w{which}{dt_i}")
                        nc.sync.dma_start(wt[:], w_ap[:, dt_i, :])
                        for s in range(NSLOT):
                            nc.tensor.matmul(
                                psums[s][:], wt[:, s * 128:(s + 1) * 128],
                                tn1_sb[:, dt_i, :],
                                start=(dt_i == 0), stop=(dt_i == DT - 1))
                    outs = []
                    for s in range(NSLOT):
                        # rope: out = q*cos + swap64(q)*sin_signed.
                        # swap64 is a cross-partition move -> PE permutation.
                        qs = atmp.tile([128, T], F32R, tag="qs")
                        nc.vector.tensor_copy(qs[:], psums[s][:])
                        swp = swps.tile([128, T], F32, tag="swp")
                        nc.tensor.matmul(swp[:], psw_sb[:], qs[:],
                                         start=True, stop=True)
                        t1 = atmp.tile([128, T], F32, tag="t1")
                        nc.vector.tensor_mul(t1[:], qs[:], cos_sb[:])
                        t2 = atmp.tile([128, T], F32, tag="t2")
                        nc.vector.tensor_mul(t2[:], swp[:], sin_sb[:])
                        o = asb.tile([128, T], F32R, tag=f"rope{which}{s}")
                        nc.vector.tensor_add(o[:], t1[:], t2[:])
                        outs.append(o)
                    qk_out.append(outs)
            qT, kT = qk_out

            # --- V (natural layout [tok, slot*128]) ---
            v_sb = []
            with tc.tile_pool(name="v_ps", bufs=4, space="PSUM") as vps:
                vp_l = [vps.tile([128, NSLOT * 128], F32, tag="vps",
                                 name=f"vp{i}") for i in range(TCH)]
                for dt_i in range(DT):
                    wvt = awp.tile([128, NSLOT * 128], F32R, tag="wqkv",
                                   name=f"wv{dt_i}")
                    nc.sync.dma_start(wvt[:], wv[:, dt_i, :])
                    for tc_i in range(TCH):
                        nc.tensor.matmul(
                            vp_l[tc_i][:],
                            tn1_sb[:, dt_i, tc_i * 128:(tc_i + 1) * 128],
                            wvt[:], start=(dt_i == 0), stop=(dt_i == DT - 1))
                for tc_i in range(TCH):
                    vs = asb.tile([128, NSLOT * 128], F32R, tag=f"v{tc_i}",
                                  name=f"v{tc_i}")
                    nc.vector.tensor_copy(vs[:], vp_l[tc_i][:])
                    v_sb.append(vs)

            # --- scores -> exp -> mask -> AV + denom, per slot ---
            ctx_n = []
            with tc.tile_pool(name="s_ps", bufs=3, space="PSUM") as sps, \
                 tc.tile_pool(name="c_ps", bufs=2, space="PSUM") as ctps, \
                 tc.tile_pool(name="dn_ps", bufs=2, space="PSUM") as dnps:
                for s in range(NSLOT):
                    ctxp = ctps.tile([128, T], F32, tag="ctx")
                    denp = dnps.tile([1, T], F32, tag="den")
                    for kc in range(TCH):
                        ncols = T - kc * 128
                        q0 = kc * 128
                        sp = sps.tile([128, T], F32, tag="scores")
                        nc.tensor.matmul(
                            sp[:, 0:ncols], kT[s][:, q0:q0 + 128], qT[s][:, q0:T],
                            start=True, stop=True)
                        ex = atmp.tile([128, T], F32R, tag="exp")
                        nc.scalar.activation(ex[:, 0:ncols], sp[:, 0:ncols], AF.Exp,
                                             scale=ISQ)
                        nc.vector.tensor_mul(ex[:, 0:ncols], ex[:, 0:ncols],
                                             mask_sb[kc][:, q0:T])
                        nc.tensor.matmul(
                            ctxp[:, q0:T], v_sb[kc][:, s * 128:(s + 1) * 128],
                            ex[:, 0:ncols], start=(kc == 0), stop=(kc == TCH - 1))
                        nc.tensor.matmul(
                            denp[:, q0:T], ones_sb[:], ex[:, 0:ncols],
                            start=(kc == 0), stop=(kc == TCH - 1))
                    rec = atmp.tile([1, T], F32, tag="rec")
                    nc.vector.reciprocal(rec[:], denp[:])
                    bc = atmp.tile([128, T], F32, tag="bc")
                    nc.gpsimd.partition_broadcast(bc[:], rec[:])
                    cn = asb.tile([128, T], F32R, tag=f"ctxn{s}")
                    nc.vector.tensor_mul(cn[:], ctxp[:], bc[:])
                    ctx_n.append(cn)

            # --- o-projection, natural [tok, d] output -> rs1_in (fp32) ---
            with tc.tile_pool(name="o_ps", bufs=2, space="PSUM") as ops_p:
                for dc in range(DCH):
                    wot = awop.tile([128, NSLOT, 512], F32R, tag="wo",
                                   name=f"wo{dc}")
                    nc.sync.dma_start(wot[:], wo[:, :, dc * 512:(dc + 1) * 512])
                    for tc_i in range(TCH):
                        op = ops_p.tile([128, 512], F32, tag="ops")
                        for s in range(NSLOT):
                            nc.tensor.matmul(
                                op[:], ctx_n[s][:, tc_i * 128:(tc_i + 1) * 128],
                                wot[:, s, :],
                                start=(s == 0), stop=(s == NSLOT - 1))
                        ob = atmp.tile([128, 512], F32, tag="ob")
                        nc.vector.tensor_copy(ob[:], op[:])
                        if dc < 3:
                            dst = rs1_in_a[:].rearrange("(c p) d -> c p d", p=128)[tc_i, :, dc * 512:(dc + 1) * 512]
                        else:
                            dst = rs1_in_b[:].rearrange("(c p) d -> c p d", p=128)[tc_i, :, (dc - 3) * 512:(dc - 2) * 512]
                        nc.sync.dma_start(dst, ob[:])

        nc.gpsimd.collective_compute(
            "ReduceScatter", ALU.add, replica_groups=[list(range(NCORES))],
            ins=[rs1_in_a.opt()], outs=[rs1_out_a.opt()])
        nc.gpsimd.collective_compute(
            "ReduceScatter", ALU.add, replica_groups=[list(range(NCORES))],
            ins=[rs1_in_b.opt()], outs=[rs1_out_b.opt()])

        hpool_cm = tc.tile_pool(name="hpool", bufs=1)
        hpool = hpool_cm.__enter__()
        mpool_cm = tc.tile_pool(name="mpool", bufs=1)
        mpool = mpool_cm.__enter__()
        h_nat = hpool.tile([TSL, D], F32, tag="h")

        # ================= Phase B: h, ln2, router, top-k =================
        with nc.named_scope("router"), \
             tc.tile_pool(name="bwork", bufs=1) as bw, \
             tc.tile_pool(name="b_ps", bufs=2, space="PSUM") as bps, \
             tc.tile_pool(name="b_ps1", bufs=1, space="PSUM") as bps1:
            o_sl = bw.tile([TSL, D], F32, tag="osl")
            nc.sync.dma_start(o_sl[:, 0:1536], rs1_out_a[:])
            nc.sync.dma_start(o_sl[:, 1536:D], rs1_out_b[:])
            x_sb = bw.tile([TSL, D], F32, tag="xsl")
            nc.sync.dma_start(x_sb[:], x_sl[:])
            nc.vector.tensor_add(h_nat[:], x_sb[:], o_sl[:])
            nc.sync.dma_start(dbg_h[:], h_nat[:])

            sq = bw.tile([TSL, D], F32, tag="sq")
            ssq = bw.tile([TSL, 1], F32, tag="ssq")
            nc.scalar.activation(sq[:], h_nat[:], AF.Square, accum_out=ssq[:])
            eps_t = bw.tile([TSL, 1], F32, tag="epst")
            nc.gpsimd.memset(eps_t[:], EPS)
            rms = bw.tile([TSL, 1], F32, tag="rms")
            nc.scalar.activation(rms[:], ssq[:], AF.Sqrt, scale=float(1.0 / D),
                                 bias=eps_t[:])
            inv = bw.tile([TSL, 1], F32, tag="inv")
            nc.vector.reciprocal(inv[:], rms[:])
            t_sl = bw.tile([TSL, D], F32, tag="tsl")
            nc.vector.tensor_scalar_mul(t_sl[:], h_nat[:], inv[:])

            # bf16 copy of normalized slice -> AllGather input
            t_sl_bf = bw.tile([TSL, D], BF16, tag="tslbf")
            nc.vector.tensor_copy(t_sl_bf[:], t_sl[:])
            nc.sync.dma_start(ag_tn_in[:], t_sl_bf[:])

            # transpose slice -> tT_cols [d, tok] f32r for the router matmul
            rwt = bw.tile([128, DT, E], F32R, tag="rw")
            nc.sync.dma_start(rwt[:], rw[:])
            tts_l = []
            for dt_i in range(DT):
                tp = bps.tile([128, TSL], F32, tag="ttp")
                nc.tensor.transpose(tp[:], t_sl[:, dt_i * 128:(dt_i + 1) * 128],
                                    id_sb[0:TSL, 0:TSL])
                tts = bw.tile([128, TSL], F32R, tag=f"tts{dt_i}")
                nc.vector.tensor_copy(tts[:], tp[:])
                tts_l.append(tts)
            lg = bps1.tile([TSL, E], F32, tag="lg")
            for dt_i in range(DT):
                nc.tensor.matmul(lg[:], tts_l[dt_i][:], rwt[:, dt_i, :],
                                 start=(dt_i == 0), stop=(dt_i == DT - 1))

            # top-k on unnormalized exp(logits): same selection and, since
            # cw = top6 / sum(top6), the softmax denominator cancels exactly.
            ex = bw.tile([TSL, E], F32, tag="exr")
            nc.scalar.activation(ex[:], lg[:], AF.Exp)
            probs = ex

            # iterative top-6: extract max 6 times
            work = bw.tile([TSL, E], F32, tag="work")
            nc.vector.tensor_copy(work[:], probs[:])
            tsum = bw.tile([TSL, 1], F32, tag="tsum")
            thr = bw.tile([TSL, 1], F32, tag="thr")
            for k in range(TOPK):
                m = bw.tile([TSL, 1], F32, tag=f"m{k}")
                nc.vector.tensor_reduce(m[:], work[:], mybir.AxisListType.X, ALU.max)
                if k == 0:
                    nc.vector.tensor_copy(tsum[:], m[:])
                else:
                    nc.vector.tensor_add(tsum[:], tsum[:], m[:])
                if k == TOPK - 1:
                    nc.vector.tensor_copy(thr[:], m[:])
                else:
                    eq = bw.tile([TSL, E], F32, tag="eq")
                    nc.vector.tensor_scalar(eq[:], work[:], m[:], 1e9,
                                            op0=ALU.is_ge, op1=ALU.mult)
                    nc.vector.tensor_sub(work[:], work[:], eq[:])
            mask6 = bw.tile([TSL, E], F32, tag="mask6")
            nc.vector.tensor_scalar(mask6[:], probs[:], thr[:], None, op0=ALU.is_ge)
            cwu = bw.tile([TSL, E], F32, tag="cwu")
            nc.vector.tensor_mul(cwu[:], probs[:], mask6[:])
            rts = bw.tile([TSL, 1], F32, tag="rts")
            nc.vector.reciprocal(rts[:], tsum[:])
            cw = bw.tile([TSL, E], F32, tag="cw")
            nc.vector.tensor_scalar_mul(cw[:], cwu[:], rts[:])
            nc.sync.dma_start(ag_cw_in[:], cw[:])

        nc.gpsimd.collective_compute(
            "AllGather", ALU.bypass, replica_groups=[list(range(NCORES))],
            ins=[ag_tn_in.opt()], outs=[ag_tn_out.opt()])
        nc.gpsimd.collective_compute(
            "AllGather", ALU.bypass, replica_groups=[list(range(NCORES))],
            ins=[ag_cw_in.opt()], outs=[ag_cw_out.opt()])
        nc.sync.dma_start(dbg_cw[:], ag_cw_out[:])

        # ================= Phase C: dispatch prep =================
        # mpool holds what phase D needs: tT, tcT, STw, (later act/down tiles)
        tT = [mpool.tile([128, T], BF16, tag=f"tT{i}", name=f"tT{i}") for i in range(DT)]
        STw = {}
        tcT = {}
        with nc.named_scope("dispatch"), \
             tc.tile_pool(name="c_sb", bufs=1) as csb, \
             tc.tile_pool(name="c_tmp", bufs=1) as ctmp:
            tnat = []
            for tc_i in range(TCH):
                tt = csb.tile([128, D], BF16, tag=f"tnat{tc_i}")
                nc.sync.dma_start(
                    tt[:], ag_tn_out[:].rearrange("(c p) d -> c p d", p=128)[tc_i])
                tnat.append(tt)
            idb = csb.tile([128, 128], BF16, tag="idb")
            nc.vector.tensor_copy(idb[:], id_sb[:])
            with tc.tile_pool(name="ct_ps", bufs=4, space="PSUM") as ctp:
                for dt_i in range(DT):
                    for tc_i in range(TCH):
                        tp = ctp.tile([128, 128], BF16, tag="ttp")
                        nc.tensor.matmul(tp[:],
                                         tnat[tc_i][:, dt_i * 128:(dt_i + 1) * 128],
                                         idb[:], is_transpose=True,
                                         start=True, stop=True)
                        nc.vector.tensor_copy(
                            tT[dt_i][:, tc_i * 128:(tc_i + 1) * 128], tp[:])

            # cw -> cwT -> my 2 experts' rows; mask/pos/posm
            with tc.tile_pool(name="cw_ps", bufs=1, space="PSUM") as cwps, \
                 tc.tile_pool(name="cw_ps2", bufs=2, space="PSUM") as cwps2:
                cwn = []
                for tc_i in range(TCH):
                    cn = ctmp.tile([128, E], F32, tag="cwn")
                    nc.sync.dma_start(
                        cn[:], ag_cw_out[:].rearrange("(c p) e -> c p e", p=128)[tc_i])
                    cwn.append(cn)
                cwTp = cwps.tile([E, T], F32, tag="cwT")
                for tc_i in range(TCH):
                    nc.tensor.matmul(cwTp[:, tc_i * 128:(tc_i + 1) * 128],
                                     cwn[tc_i][:], id_sb[:], is_transpose=True,
                                     start=True, stop=True)
                cwT = csb.tile([E, T], F32R, tag="cwTs")
                nc.vector.tensor_copy(cwT[:], cwTp[:])
                selt = csb.tile([16, EPC], F32R, tag="sel")
                nc.sync.dma_start(selt[:], sel[:])
                zz = csb.tile([1, T], F32, tag="zz")
                nc.gpsimd.memset(zz[:], 0.0)
                pn = [csb.tile([128, EPC], F32, tag=f"pn{i}", name=f"pn{i}")
                      for i in range(TCH)]
                cwm = []
                posm = []
                for e in range(EPC):
                    # this expert's coefficient row, at partition 0
                    cwmp = cwps.tile([1, T], F32, tag=f"cwm{e}", name=f"cwmp{e}")
                    nc.tensor.matmul(cwmp[:], selt[:, e:e + 1], cwT[:],
                                     start=True, stop=True)
                    cwm_e = csb.tile([1, T], F32, tag=f"cwms{e}", name=f"cwms{e}")
                    nc.vector.tensor_copy(cwm_e[:], cwmp[:])
                    cwm.append(cwm_e)
                    msk = csb.tile([1, T], F32, tag=f"msk{e}", name=f"msk{e}")
                    nc.vector.tensor_scalar(msk[:], cwm_e[:], 0.0, None,
                                            op0=ALU.is_gt)
                    pos = csb.tile([1, T], F32, tag=f"pos{e}", name=f"pos{e}")
                    nc.vector.tensor_tensor_scan(pos[:], msk[:], zz[:], 0.0,
                                                 op0=ALU.add, op1=ALU.add)
                    pm = csb.tile([1, T], F32, tag=f"posm{e}", name=f"posm{e}")
                    nc.vector.tensor_mul(pm[:], pos[:], msk[:])
                    nc.vector.tensor_scalar_add(pm[:], pm[:], -1.0)
                    posm.append(pm)
                    # posm_nat [tok, 1] per token chunk -> pn[tc][:, e]
                    for tc_i in range(TCH):
                        pp = cwps2.tile([128, 1], F32, tag="pn")
                        nc.tensor.matmul(pp[:],
                                         pm[:, tc_i * 128:(tc_i + 1) * 128],
                                         id_sb[0:1, 0:1], is_transpose=True,
                                         start=True, stop=True)
                        nc.vector.tensor_copy(pn[tc_i][:, e:e + 1], pp[:])

            # S (gather) and cw-weighted ST (scatter) one-hot matrices
            iC = csb.tile([128, C], F32, tag="iC")
            nc.sync.dma_start(iC[:], iotaC[:])
            iS = csb.tile([128, SC], F32, tag="iS")
            nc.sync.dma_start(iS[:], iotaS[:])
            S = {}
            for e in range(EPC):
                for tc_i in range(TCH):
                    st = csb.tile([128, C], BF16, tag=f"S{e}_{tc_i}")
                    nc.vector.tensor_scalar(st[:], iC[:], pn[tc_i][:, e:e + 1], None,
                                            op0=ALU.is_equal)
                    S[(e, tc_i)] = st
                pb = ctmp.tile([128, T], F32, tag="pb")
                nc.gpsimd.partition_broadcast(pb[:], posm[e][:])
                cb = ctmp.tile([128, T], F32, tag="cb")
                nc.gpsimd.partition_broadcast(cb[:], cwm[e][:])
                for sc_i in range(SC):
                    t1 = ctmp.tile([128, T], F32, tag="st1")
                    nc.vector.tensor_scalar(t1[:], pb[:], iS[:, sc_i:sc_i + 1], None,
                                            op0=ALU.is_equal)
                    stw = mpool.tile([128, T], BF16, tag=f"STw{e}_{sc_i}")
                    nc.vector.tensor_mul(stw[:], t1[:], cb[:])
                    STw[(e, sc_i)] = stw

            # gather: tcT[e] tiles [128(d), C] bf16
            with tc.tile_pool(name="g_ps", bufs=3, space="PSUM") as gps_p:
                for e in range(EPC):
                    for dt_i in range(DT):
                        gp = gps_p.tile([128, C], F32, tag="gps")
                        for tc_i in range(TCH):
                            nc.tensor.matmul(
                                gp[:], tnat[tc_i][:, dt_i * 128:(dt_i + 1) * 128],
                                S[(e, tc_i)][:],
                                start=(tc_i == 0), stop=(tc_i == TCH - 1))
                        g = mpool.tile([128, C], BF16, tag=f"tcT{e}_{dt_i}")
                        nc.vector.tensor_copy(g[:], gp[:])
                        tcT[(e, dt_i)] = g

        # ================= Phase D: experts =================
        act = {}
        act_sh = []
        down = {}
        with nc.named_scope("experts"), \
             tc.tile_pool(name="d_wd", bufs=3) as dwdp, \
             tc.tile_pool(name="d_sb", bufs=2) as dsb, \
             tc.tile_pool(name="gu_ps", bufs=2, space="PSUM") as gups, \
             tc.tile_pool(name="dn_ps", bufs=4, space="PSUM") as dnps:
            # per expert: gate/up -> act, then down (overlaps next expert's
            # gate/up through independent psum pools)
            for e in range(EPC):
                for it in range(IT):
                    wgt = dwp.tile([128, DT * 128], BF16, tag="wg")
                    nc.gpsimd.dma_start(wgt[:], wg[e, it])
                    gp = gups.tile([128, T], F32, tag="gps")
                    for dt_i in range(DT):
                        nc.tensor.matmul(gp[:, 0:C],
                                         wgt[:, dt_i * 128:(dt_i + 1) * 128],
                                         tcT[(e, dt_i)][:],
                                         start=(dt_i == 0), stop=(dt_i == DT - 1))
                    wut = dwp.tile([128, DT * 128], BF16, tag="wu")
                    nc.gpsimd.dma_start(wut[:], wu[e, it])
                    up = gups.tile([128, T], F32, tag="ups")
                    for dt_i in range(DT):
                        nc.tensor.matmul(up[:, 0:C],
                                         wut[:, dt_i * 128:(dt_i + 1) * 128],
                                         tcT[(e, dt_i)][:],
                                         start=(dt_i == 0), stop=(dt_i == DT - 1))
                    sl = dsb.tile([128, C], F32, tag="sl")
                    nc.scalar.activation(sl[:], gp[:, 0:C], AF.Silu)
                    a = mpool.tile([128, C], BF16, tag=f"act{e}_{it}")
                    nc.vector.tensor_mul(a[:], sl[:], up[:, 0:C])
                    act[(e, it)] = a
                for dc in range(DCH):
                    wdt = dwdp.tile([128, IT * 512], BF16, tag="wd")
                    nc.gpsimd.dma_start(wdt[:], wd[e, dc])
                    for sc_i in range(SC):
                        dp = dnps.tile([128, 512], F32, tag="dn")
                        for it in range(IT):
                            nc.tensor.matmul(
                                dp[:],
                                act[(e, it)][:, sc_i * 128:(sc_i + 1) * 128],
                                wdt[:, it * 512:(it + 1) * 512],
                                start=(it == 0), stop=(it == IT - 1))
                        db = mpool.tile([128, 512], BF16, tag=f"db{e}_{sc_i}_{dc}")
                        nc.vector.tensor_copy(db[:], dp[:])
                        down[(e, sc_i, dc)] = db

            # shared expert gate/up -> act_sh
            for it in range(SIT):
                sgt = dwp.tile([128, DT * 128], BF16, tag="wg")
                nc.gpsimd.dma_start(sgt[:], swg[it])
                gp = gups.tile([128, T], F32, tag="gps")
                for dt_i in range(DT):
                    nc.tensor.matmul(gp[:], sgt[:, dt_i * 128:(dt_i + 1) * 128],
                                     tT[dt_i][:],
                                     start=(dt_i == 0), stop=(dt_i == DT - 1))
                sut = dwp.tile([128, DT * 128], BF16, tag="wu")
                nc.gpsimd.dma_start(sut[:], swu[it])
                up = gups.tile([128, T], F32, tag="ups")
                for dt_i in range(DT):
                    nc.tensor.matmul(up[:], sut[:, dt_i * 128:(dt_i + 1) * 128],
                                     tT[dt_i][:],
                                     start=(dt_i == 0), stop=(dt_i == DT - 1))
                sl = dsb.tile([128, T], F32, tag="ssl")
                nc.scalar.activation(sl[:], gp[:], AF.Silu)
                a = mpool.tile([128, T], BF16, tag=f"acts{it}")
                nc.vector.tensor_mul(a[:], sl[:], up[:])
                act_sh.append(a)

        # final: scatter routed + shared down -> rs2_in [T, D] bf16
        with nc.named_scope("combine"), \
             tc.tile_pool(name="f_w", bufs=1) as fwp, \
             tc.tile_pool(name="f_sb", bufs=3) as fsb, \
             tc.tile_pool(name="f_ps", bufs=3, space="PSUM") as fps_p:
            swd_sb = []
            for it in range(SIT):
                sdt = fwp.tile([128, D], BF16, tag=f"swd{it}")
                nc.gpsimd.dma_start(sdt[:], swd[it])
                swd_sb.append(sdt)
            for tc_i in range(TCH):
                for dc in range(DCH):
                    fp = fps_p.tile([128, 512], F32, tag="fps")
                    for it in range(SIT):
                        nc.tensor.matmul(
                            fp[:], act_sh[it][:, tc_i * 128:(tc_i + 1) * 128],
                            swd_sb[it][:, dc * 512:(dc + 1) * 512],
                            start=(it == 0), stop=False)
                    n_sc = EPC * SC
                    cnt = 0
                    for e in range(EPC):
                        for sc_i in range(SC):
                            cnt += 1
                            nc.tensor.matmul(
                                fp[:],
                                STw[(e, sc_i)][:, tc_i * 128:(tc_i + 1) * 128],
                                down[(e, sc_i, dc)][:],
                                start=False, stop=(cnt == n_sc))
                    fb = fsb.tile([128, 512], BF16, tag="fb")
                    nc.vector.tensor_copy(fb[:], fp[:])
                    nc.sync.dma_start(
                        rs2_in[:].rearrange("(c p) d -> c p d", p=128)[tc_i, :, dc * 512:(dc + 1) * 512],
                        fb[:])

        nc.gpsimd.collective_compute(
            "ReduceScatter", ALU.add, replica_groups=[list(range(NCORES))],
            ins=[rs2_in.opt()], outs=[rs2_out.opt()])

        # ================= Final: residual add =================
        with tc.tile_pool(name="fin", bufs=1) as fin:
            moe_bf = fin.tile([TSL, D], BF16, tag="moebf")
            nc.sync.dma_start(moe_bf[:], rs2_out[:])
            moe_f = fin.tile([TSL, D], F32, tag="moef")
            nc.vector.tensor_copy(moe_f[:], moe_bf[:])
            o = fin.tile([TSL, D], F32, tag="o")
            nc.vector.tensor_add(o[:], h_nat[:], moe_f[:])
            nc.sync.dma_start(out_sl[:], o[:])

        mpool_cm.__exit__(None, None, None)
        hpool_cm.__exit__(None, None, None)

    nc.compile()
    return nc


def _prep_inputs(inputs):
    """Host-side prep: norms/tables/layout/sharding. Returns in_maps[8]."""
    f32 = np.float32
    x = np.asarray(inputs["hidden_states"], dtype=f32).reshape(T, D)
    ln1w = np.asarray(inputs["ln1_w"], dtype=f32)
    ln2w = np.asarray(inputs["ln2_w"], dtype=f32)
    pos_ids = np.asarray(inputs["position_ids"]).reshape(T)
    amask = np.asarray(inputs["attention_mask"]).reshape(T, T)

    xd = x.astype(np.float64)
    inv1 = 1.0 / np.sqrt((xd * xd).mean(axis=1, keepdims=True) + EPS)
    tn1 = ((xd * inv1).astype(f32)) * ln1w[None, :]
    tn1T_t = np.ascontiguousarray(
        tn1.T.reshape(DT, 128, T).transpose(1, 0, 2)).astype(f32)

    inv_freq = 1.0 / (5e6 ** (np.arange(0, HD, 2, dtype=f32) / HD))
    ang = pos_ids.astype(f32)[:, None] * inv_freq[None, :]        # [T, 64]
    cos_f = np.concatenate([np.cos(ang), np.cos(ang)], axis=1)    # [T, 128]
    sin_h = np.sin(ang)
    sinT_h = np.concatenate([-sin_h, sin_h], axis=1).T            # [128, T]
    cosT_h = np.ascontiguousarray(cos_f.T).astype(f32)
    sinT_h = np.ascontiguousarray(sinT_h).astype(f32)

    causal = np.tril(np.ones((T, T), dtype=bool))
    mk = (amask & causal).T.astype(f32)                           # [k, q]
    maskT_h = np.ascontiguousarray(mk.reshape(TCH, 128, T)).astype(f32)

    wq_f = np.asarray(inputs["wq"], dtype=f32)
    wk_f = np.asarray(inputs["wk"], dtype=f32)
    wv_f = np.asarray(inputs["wv"], dtype=f32)
    wo_f = np.asarray(inputs["wo"], dtype=f32)
    rw_f = np.asarray(inputs["router_w"], dtype=f32) * ln2w[:, None]
    wg_f = np.asarray(inputs["w_gate"], dtype=f32) * ln2w[None, :, None]
    wu_f = np.asarray(inputs["w_up"], dtype=f32) * ln2w[None, :, None]
    wd_f = np.asarray(inputs["w_down"], dtype=f32)
    sg_f = np.asarray(inputs["sw_gate"], dtype=f32) * ln2w[:, None]
    su_f = np.asarray(inputs["sw_up"], dtype=f32) * ln2w[:, None]
    sd_f = np.asarray(inputs["sw_down"], dtype=f32)

    rw_t = np.ascontiguousarray(rw_f.reshape(DT, 128, E).transpose(1, 0, 2))

    head_of = []
    for c in range(NCORES):
        if c < 4:
            head_of.append([3 * c, 3 * c + 1, 3 * c + 2])
        else:
            head_of.append([12 + 2 * (c - 4), 13 + 2 * (c - 4), None])

    iotaC_h = np.broadcast_to(np.arange(C, dtype=f32), (128, C)).copy()
    iotaS_h = (np.arange(128, dtype=f32)[:, None]
               + 128.0 * np.arange(SC, dtype=f32)[None, :]).copy()
    ident_h = np.eye(128, dtype=f32)
    ones_h = np.ones((128, 1), dtype=f32)
    # pswap[d, j] = 1 iff d == (j + 64) % 128, so (P.T @ q)[j] = q[(j+64)%128]
    pswap_h = np.zeros((128, 128), dtype=f32)
    for j in range(128):
        pswap_h[(j + 64) % 128, j] = 1.0

    sip = 3328 // NCORES  # 416

    def tile_gate(w):  # [D, SI] -> [SIT, 128, DT*128]
        return np.ascontiguousarray(
            w.reshape(DT, 128, SIT, 128).transpose(2, 1, 0, 3).reshape(
                SIT, 128, DT * 128))

    def exp_gate(w2):  # [D, I] -> [IT, 128, DT*128]
        return np.ascontiguousarray(
            w2.reshape(DT, 128, IT, 128).transpose(2, 1, 0, 3).reshape(
                IT, 128, DT * 128))

    def exp_down(w2):  # [I, D] -> [DCH, 128, IT*512]
        return np.ascontiguousarray(
            w2.reshape(IT, 128, DCH, 512).transpose(2, 1, 0, 3).reshape(
                DCH, 128, IT * 512))

    in_maps = []
    for c in range(NCORES):
        hs = head_of[c]
        wq_c = np.zeros((D, NSLOT * 128), f32)
        wk_c = np.zeros((D, NSLOT * 128), f32)
        wv_c = np.zeros((D, NSLOT * 128), f32)
        wo_c = np.zeros((NSLOT * 128, D), f32)
        for s, h in enumerate(hs):
            if h is None:
                continue
            wq_c[:, s * 128:(s + 1) * 128] = wq_f[:, h * 128:(h + 1) * 128]
            wk_c[:, s * 128:(s + 1) * 128] = wk_f[:, h * 128:(h + 1) * 128]
            wv_c[:, s * 128:(s + 1) * 128] = wv_f[:, h * 128:(h + 1) * 128]
            wo_c[s * 128:(s + 1) * 128, :] = wo_f[h * 128:(h + 1) * 128, :]

        def qt(w):  # [D, 384] -> [128, DT, 384]
            return np.ascontiguousarray(
                w.reshape(DT, 128, NSLOT * 128).transpose(1, 0, 2))

        wo_t = np.ascontiguousarray(wo_c.reshape(NSLOT, 128, D).transpose(1, 0, 2))

        e0, e1 = 2 * c, 2 * c + 1
        sel_h = np.zeros((16, EPC), f32)
        sel_h[e0, 0] = 1.0
        sel_h[e1, 1] = 1.0

        wg_c = np.stack([exp_gate(wg_f[e0]), exp_gate(wg_f[e1])]).astype(ml_bf16)
        wu_c = np.stack([exp_gate(wu_f[e0]), exp_gate(wu_f[e1])]).astype(ml_bf16)
        wd_c = np.stack([exp_down(wd_f[e0]), exp_down(wd_f[e1])]).astype(ml_bf16)

        s0 = c * sip
        sg_c = np.zeros((D, SI), f32)
        su_c = np.zeros((D, SI), f32)
        sd_c = np.zeros((SI, D), f32)
        sg_c[:, :sip] = sg_f[:, s0:s0 + sip]
        su_c[:, :sip] = su_f[:, s0:s0 + sip]
        sd_c[:sip, :] = sd_f[s0:s0 + sip, :]

        in_maps.append({
            "tn1T": tn1T_t,
            "x_sl": np.ascontiguousarray(x[c * TSL:(c + 1) * TSL]),
            "wq": qt(wq_c), "wk": qt(wk_c), "wv": qt(wv_c), "wo": wo_t,
            "cosT": cosT_h, "sinT": sinT_h, "maskT": maskT_h,
            "rw": rw_t, "sel": sel_h, "ones": ones_h, "ident": ident_h,
            "pswap": pswap_h,
            "iotaC": iotaC_h, "iotaS": iotaS_h,
            "wg": wg_c, "wu": wu_c, "wd": wd_c,
            "swg": tile_gate(sg_c).astype(ml_bf16),
            "swu": tile_gate(su_c).astype(ml_bf16),
            "swd": sd_c.reshape(SIT, 128, D).astype(ml_bf16),
        })
    return in_maps


def kernel(**inputs) -> np.ndarray:
    if "nc" not in _CACHE:
        _CACHE["nc"] = _build()
    nc = _CACHE["nc"]
    in_maps = _prep_inputs(inputs)
    res = run_bass_kernel_spmd(nc, in_maps, core_ids=list(range(NCORES)), trace=TRACE)
    _CACHE["last_results"] = res
    out = np.concatenate([res.results[c]["out_sl"] for c in range(NCORES)], axis=0)
    return out.reshape(1, T, D).astype(np.float32)
